# revision 1
# baseline (speedup 1.0000x reference)
"""Trainium2 Bass kernel for dual-branch (hifi windowed + lofi downsampled-KV)
attention. Data-parallel over batch: 8 batches -> 8 NeuronCores.

Per batch (x: (512, 4096) channel-major):
  hifi: q,k = Whqkv[:512]@x+b (channels on partitions); V^T produced directly
        (keys on partitions) by a transposed projection with a ones-column for
        softmax denominators. Window (4 consecutive row-major pixels) masking
        is folded into the logits matmul via an extra 33-row contraction
        (onehot-group outer product: adds 0 in-window, -320 out-of-window,
        pre-scale). exp on ACT (scale fused), attn@V on PE, denominators
        packed via DMA -> reciprocal_approx_fast -> partition_broadcast,
        normalize fused with the (g,i)->(h,w) output scatter permutation.
  lofi: q (4096), k,v from 2x2-avgpooled x (1024 keys; pooling = 2 strided
        adds, the /4 folded into W_lkv). S^T with keys on partitions, chunked
        by 128 keys; exp on ACT; attn@V accumulates over chunks with a
        ones-column for denominators; same normalize pipeline.
  Output: y[0:256] = Whproj@hifi+b, y[256:512] = Wlproj@lofi+b.
"""
import sys

sys.path.insert(0, "/opt/trn_rl_repo")

import numpy as np
import ml_dtypes

import concourse.bass as bass
import concourse.bacc as bacc
import concourse.mybir as mybir
import concourse.tile as tile
from concourse.bass_utils import run_bass_kernel_spmd

F32 = mybir.dt.float32
F32R = mybir.dt.float32r
BF16 = mybir.dt.bfloat16
AF = mybir.ActivationFunctionType

SCALE = 64 ** -0.5   # 0.125
N = 4096
M = 1024
CBIG = 320.0         # mask magnitude pre-scale (C/SCALE with C=40)

_CACHE = {}


def _build_bass():
    nc = bacc.Bacc("TRN2", target_bir_lowering=False, debug=False, num_devices=8)

    d = {}
    d["x_d"] = nc.dram_tensor("x", (512, N), BF16, kind="ExternalInput").ap()
    d["xf_d"] = nc.dram_tensor("xf", (512, N), F32, kind="ExternalInput").ap()
    for nm, shp, dt in [
        ("wqk", (512, 512), BF16), ("bqk", (128, 4), F32),
        ("whv", (512, 260), BF16), ("bhv", (1, 260), BF16),
        ("wlq", (512, 256), BF16), ("blq", (128, 2), F32),
        ("wlk", (512, 256), BF16), ("blk", (128, 2), F32),
        ("wlv", (512, 260), BF16), ("blv", (1, 260), BF16),
        ("whp", (64, 1024), BF16), ("bhp", (128, 2), F32),
        ("wlp", (64, 1024), BF16), ("blp", (128, 2), F32),
        ("mskL", (128, 128), BF16), ("mskR", (128, 128), BF16),
    ]:
        d[nm + "_d"] = nc.dram_tensor(nm, shp, dt, kind="ExternalInput").ap()
    d["y_d"] = nc.dram_tensor("y", (512, N), F32, kind="ExternalOutput").ap()

    with tile.TileContext(nc) as tc:
        _emit(nc, tc, d)
    nc.finalize()
    return nc


def _emit(nc, tc, d):
    import os
    PH = os.environ.get("KPHASES", "1hHlL")
    HSUB = int(os.environ.get("HSUB", "9"))
    x_d = d["x_d"]; y_d = d["y_d"]

    # ---- persistent: weights ----
    wp = tc.alloc_tile_pool(name="wp", bufs=1)
    wqk = wp.tile([128, 512 * 4], BF16, tag="wqk", name="wqk")
    bqk = wp.tile([128, 4], F32, tag="bqk", name="bqk")
    whv = wp.tile([128, 260 * 4], BF16, tag="whv", name="whv")
    bhv = wp.tile([1, 260], BF16, tag="bhv", name="bhv")
    wlq = wp.tile([128, 256 * 4], BF16, tag="wlq", name="wlq")
    blq = wp.tile([128, 2], F32, tag="blq", name="blq")
    wlk = wp.tile([128, 256 * 4], BF16, tag="wlk", name="wlk")
    blk = wp.tile([128, 2], F32, tag="blk", name="blk")
    wlv = wp.tile([128, 260 * 4], BF16, tag="wlv", name="wlv")
    blv = wp.tile([1, 260], BF16, tag="blv", name="blv")
    whp = wp.tile([64, 1024], BF16, tag="whp", name="whp")
    bhp = wp.tile([128, 2], F32, tag="bhp", name="bhp")
    wlp = wp.tile([64, 1024], BF16, tag="wlp", name="wlp")
    blp = wp.tile([128, 2], F32, tag="blp", name="blp")
    mskL = wp.tile([128, 128], BF16, tag="mskL", name="mskL")
    mskR = wp.tile([128, 128], BF16, tag="mskR", name="mskR")
    ones1 = wp.tile([1, 128], BF16, tag="ones1", name="ones1")

    for (t, nm) in [(wqk, "wqk"), (whv, "whv"), (wlq, "wlq"), (wlk, "wlk"), (wlv, "wlv")]:
        dr = d[nm + "_d"]
        w = dr.shape[1]
        for kt in range(4):
            nc.sync.dma_start(t[:, kt * w:(kt + 1) * w], dr[kt * 128:(kt + 1) * 128, :])
    for (t, nm) in [(whp, "whp"), (wlp, "wlp")]:
        nc.sync.dma_start(t[:], d[nm + "_d"][:, :])
    for (t, nm) in [(bqk, "bqk"), (bhv, "bhv"), (blq, "blq"), (blk, "blk"),
                    (blv, "blv"), (bhp, "bhp"), (blp, "blp"),
                    (mskL, "mskL"), (mskR, "mskR")]:
        nc.sync.dma_start(t[:], d[nm + "_d"][:, :])
    nc.vector.memset(ones1[:], 1.0)

    # ---- persistent pools, ordered by release time (LIFO stack) ----
    opl = tc.alloc_tile_pool(name="oplofi", bufs=1)       # lives until after lofi
    lq = [opl.tile([128, N], BF16, tag=f"lq{mt}", name=f"lq{mt}") for mt in range(2)]
    lk = [opl.tile([128, M], BF16, tag=f"lk{mt}", name=f"lk{mt}") for mt in range(2)]
    val = opl.tile([128, 260 * 8], BF16, tag="val", name="val")
    oph = tc.alloc_tile_pool(name="ophifi", bufs=1)       # lives until after hifi proj
    qkh = [oph.tile([128, N], BF16, tag=f"qkh{mt}", name=f"qkh{mt}") for mt in range(4)]
    vah = oph.tile([128, 260 * 32], BF16, tag="vah", name="vah")
    opx = tc.alloc_tile_pool(name="opx", bufs=1)          # lives through P1 only
    xp = [opx.tile([128, M], F32, tag=f"xp{kt}", name=f"xp{kt}") for kt in range(4)]

    # =================== P1: projections ===================
    with tc.tile_pool(name="p1x", bufs=3) as xin_p, \
         tc.tile_pool(name="p1t1", bufs=3) as t1_p, \
         tc.tile_pool(name="p1ps", bufs=4, space="PSUM") as ps_p, \
         tc.tile_pool(name="p1ps2", bufs=4, space="PSUM") as ps2_p:
        for nt in range(8):
            xb = [xin_p.tile([128, 512], BF16, tag=f"xb{kt}", name=f"xb{kt}") for kt in range(4)]
            xbf = [xin_p.tile([128, 512], F32, tag=f"xbf{kt}", name=f"xbf{kt}") for kt in range(4)]
            for kt in range(4):
                nc.sync.dma_start(xb[kt][:], x_d[kt * 128:(kt + 1) * 128, nt * 512:(nt + 1) * 512])
                nc.sync.dma_start(xbf[kt][:], d["xf_d"][kt * 128:(kt + 1) * 128, nt * 512:(nt + 1) * 512])
            # pooling (8 h-rows per block): w-pairs then h-pairs
            for kt in range(4):
                v = xbf[kt][:].rearrange("p (h w2 two) -> p h w2 two", w2=32, two=2)
                t1 = t1_p.tile([128, 256], F32, tag="t1", name="t1")
                t1v = t1[:].rearrange("p (h w2) -> p h w2", w2=32)
                nc.gpsimd.tensor_add(t1v, v[:, :, :, 0], v[:, :, :, 1])
                t1p = t1[:].rearrange("p (i two w2) -> p i two w2", two=2, w2=32)
                xpv = xp[kt][:, nt * 128:(nt + 1) * 128].rearrange("p (i w2) -> p i w2", w2=32)
                nc.gpsimd.tensor_add(xpv, t1p[:, :, 0, :], t1p[:, :, 1, :])
            # hifi q,k
            for mt in range(4):
                ps = ps_p.tile([128, 512], F32, tag="ps", name="ps")
                for kt in range(4):
                    nc.tensor.matmul(ps[:], wqk[:, kt * 512 + mt * 128: kt * 512 + (mt + 1) * 128],
                                     xb[kt][:], start=(kt == 0), stop=(kt == 3))
                nc.scalar.activation(qkh[mt][:, nt * 512:(nt + 1) * 512], ps[:],
                                     AF.Identity, bias=bqk[:, mt:mt + 1], scale=1.0)
            # hifi V^T aug
            for sc in range(4):
                st = nt * 4 + sc
                ps = ps2_p.tile([128, 260], F32, tag="psv", name="psv")
                for kt in range(4):
                    nc.tensor.matmul(ps[:], xb[kt][:, sc * 128:(sc + 1) * 128],
                                     whv[:, kt * 260:(kt + 1) * 260], start=(kt == 0), stop=False)
                nc.tensor.matmul(ps[:], ones1[:], bhv[:], start=False, stop=True)
                nc.vector.tensor_copy(vah[:, st * 260:(st + 1) * 260], ps[:])
            # lofi q
            for mt in range(2):
                ps = ps_p.tile([128, 512], F32, tag="ps", name="ps")
                for kt in range(4):
                    nc.tensor.matmul(ps[:], wlq[:, kt * 256 + mt * 128: kt * 256 + (mt + 1) * 128],
                                     xb[kt][:], start=(kt == 0), stop=(kt == 3))
                nc.scalar.activation(lq[mt][:, nt * 512:(nt + 1) * 512], ps[:],
                                     AF.Identity, bias=blq[:, mt:mt + 1], scale=1.0)
        # convert pooled x to bf16 for the kv projections
        xpb = [opx.tile([128, M], BF16, tag=f"xpb{kt}", name=f"xpb{kt}") for kt in range(4)]
        for kt in range(4):
            nc.vector.tensor_copy(xpb[kt][:], xp[kt][:])
        # lofi k
        for mt in range(2):
            for ntk in range(2):
                ps = ps_p.tile([128, 512], F32, tag="ps", name="ps")
                for kt in range(4):
                    nc.tensor.matmul(ps[:], wlk[:, kt * 256 + mt * 128: kt * 256 + (mt + 1) * 128],
                                     xpb[kt][:, ntk * 512:(ntk + 1) * 512], start=(kt == 0), stop=(kt == 3))
                nc.scalar.activation(lk[mt][:, ntk * 512:(ntk + 1) * 512], ps[:],
                                     AF.Identity, bias=blk[:, mt:mt + 1], scale=1.0)
        # lofi V^T aug
        for mc in range(8):
            ps = ps2_p.tile([128, 260], F32, tag="psv", name="psv")
            for kt in range(4):
                nc.tensor.matmul(ps[:], xpb[kt][:, mc * 128:(mc + 1) * 128],
                                 wlv[:, kt * 260:(kt + 1) * 260], start=(kt == 0), stop=False)
            nc.tensor.matmul(ps[:], ones1[:], blv[:], start=False, stop=True)
            nc.vector.tensor_copy(val[:, mc * 260:(mc + 1) * 260], ps[:])

    opx.release()

    # ---- per-head normalized hifi outputs (bf16, rows 0..63) ----
    nbh = tc.alloc_tile_pool(name="normbufh", bufs=1)
    norm_h = [nbh.tile([64, N], BF16, tag=f"nh{h}", name=f"nh{h}") for h in range(4)]

    # =================== hifi attention ===================
    if "h" in PH:
      with tc.tile_pool(name="hL", bufs=2, space="PSUM") as hL_p, \
           tc.tile_pool(name="hO", bufs=2, space="PSUM") as hO_p, \
           tc.tile_pool(name="hexp", bufs=4) as he_p, \
           tc.tile_pool(name="hstg", bufs=4) as hs_p, \
           tc.tile_pool(name="hpk", bufs=6) as hpk_p, \
           tc.tile_pool(name="hrb", bufs=2) as hrb_p:
          for h in range(4):
              qt_ = qkh[h // 2]
              kt_ = qkh[2 + h // 2]
              roff = 64 * (h % 2)
              for sg in range(4):
                  Lg = hL_p.tile([128, 1024], F32, tag="Lg", name="Lg")
                  for s4 in range(8):
                      st = sg * 8 + s4
                      nc.tensor.matmul(Lg[:, s4 * 128:(s4 + 1) * 128],
                                       kt_[roff:roff + 64, st * 128:(st + 1) * 128],
                                       qt_[roff:roff + 64, st * 128:(st + 1) * 128],
                                       start=True, stop=(HSUB == 0))
                      if HSUB != 0:
                          nc.tensor.matmul(Lg[:, s4 * 128:(s4 + 1) * 128],
                                           mskL[roff:roff + 64, :], mskR[roff:roff + 64, :],
                                           start=False, stop=True)
                  eL = he_p.tile([128, 1024], BF16, tag="eL", name="eL")
                  nc.scalar.activation(eL[:], Lg[:], AF.Exp, scale=SCALE)
                  if HSUB < 2:
                      continue
                  oh = hO_p.tile([128, 1024], F32, tag="oh", name="oh")
                  for s4 in range(8):
                      st = sg * 8 + s4
                      nc.tensor.matmul(oh[0:65, s4 * 128:(s4 + 1) * 128],
                                       vah[:, st * 260 + h * 65: st * 260 + (h + 1) * 65],
                                       eL[:, s4 * 128:(s4 + 1) * 128], start=True, stop=True)
                  stg = hs_p.tile([65, 1024], F32, tag="stg", name="stg")
                  nc.vector.tensor_copy(stg[:], oh[0:65, :])
                  if HSUB < 3:
                      continue
                  dpk = hpk_p.tile([128, 8], F32, tag="dpk", name="dpk")
                  nc.sync.dma_start(dpk[:], stg[64:65, :].rearrange("o (p f) -> o p f", f=8))
                  rpk = hpk_p.tile([128, 8], F32, tag="rpk", name="rpk")
                  nc.vector.reciprocal_approx_fast(rpk[:], dpk[:])
                  rrow = hpk_p.tile([1, 1024], F32, tag="rrow", name="rrow")
                  nc.sync.dma_start(rrow[:].rearrange("o (p f) -> o p f", f=8), rpk[:])
                  rb = hrb_p.tile([64, 1024], F32, tag="rb", name="rb")
                  nc.gpsimd.partition_broadcast(rb[:], rrow[:])
                  # normalize + scatter: src col (s8,g,i1,i2) -> dst 128*s8+64*i1+2*g+i2
                  # split by i1 to keep APs within 3 free dims
                  dstv = norm_h[h][:, sg * 1024:(sg + 1) * 1024].rearrange(
                      "p (s i1 gi2) -> p s i1 gi2", s=8, i1=2, gi2=64)
                  srcv = stg[0:64, :].rearrange("p (s g i1 i2) -> p s g i1 i2", s=8, g=32, i1=2, i2=2)
                  rbv = rb[:].rearrange("p (s g i1 i2) -> p s g i1 i2", s=8, g=32, i1=2, i2=2)
                  for i1 in range(2):
                      nc.vector.tensor_tensor(dstv[:, :, i1, :], srcv[:, :, :, i1, :], rbv[:, :, :, i1, :], mybir.AluOpType.mult)

    # =================== hifi projection -> y[0:256] ===================
    if "H" in PH:
      with tc.tile_pool(name="hpps", bufs=4, space="PSUM") as pps, \
           tc.tile_pool(name="hpyb", bufs=4) as yb_p:
          for mt in range(2):
              for nt in range(8):
                  ps = pps.tile([128, 512], F32, tag="ps", name="ps")
                  for h in range(4):
                      nc.tensor.matmul(ps[:],
                                       whp[:, h * 256 + mt * 128: h * 256 + (mt + 1) * 128],
                                       norm_h[h][:, nt * 512:(nt + 1) * 512],
                                       start=(h == 0), stop=(h == 3))
                  yb = yb_p.tile([128, 512], F32, tag="yb", name="yb")
                  nc.scalar.activation(yb[:], ps[:], AF.Identity, bias=bhp[:, mt:mt + 1], scale=1.0)
                  nc.sync.dma_start(y_d[mt * 128:(mt + 1) * 128, nt * 512:(nt + 1) * 512], yb[:])

    nbh.release()
    oph.release()

    # ---- per-head normalized lofi outputs ----
    nbl = tc.alloc_tile_pool(name="normbufl", bufs=1)
    norm_l = [nbl.tile([64, N], BF16, tag=f"nl{h}", name=f"nl{h}") for h in range(4)]

    # =================== lofi attention ===================
    if "l" in PH:
      with tc.tile_pool(name="lS", bufs=2, space="PSUM") as lS_p, \
           tc.tile_pool(name="lO", bufs=1, space="PSUM") as lO_p, \
           tc.tile_pool(name="lexp", bufs=4) as le_p, \
           tc.tile_pool(name="lstg", bufs=3) as ls_p, \
           tc.tile_pool(name="lpk", bufs=4) as lpk_p, \
           tc.tile_pool(name="lrb", bufs=2) as lrb_p:
          for h in range(4):
              roff = 64 * (h % 2)
              for qg in range(2):
                  qb = qg * 2048
                  oaccs = [lO_p.tile([128, 512], F32, tag=f"oac{i}", name=f"oac{i}") for i in range(4)]
                  for c in range(8):
                      for hf in range(2):
                          sg = lS_p.tile([128, 1024], F32, tag="sg", name="sg")
                          for q2 in range(2):
                              qt = hf * 2 + q2
                              nc.tensor.matmul(sg[:, q2 * 512:(q2 + 1) * 512],
                                               lk[h // 2][roff:roff + 64, c * 128:(c + 1) * 128],
                                               lq[h // 2][roff:roff + 64, qb + qt * 512: qb + (qt + 1) * 512],
                                               start=True, stop=True)
                          eS = le_p.tile([128, 1024], BF16, tag="eS", name="eS")
                          nc.scalar.activation(eS[:], sg[:], AF.Exp, scale=SCALE)
                          for q2 in range(2):
                              qt = hf * 2 + q2
                              nc.tensor.matmul(oaccs[qt][0:65, :],
                                               val[:, c * 260 + h * 65: c * 260 + (h + 1) * 65],
                                               eS[:, q2 * 512:(q2 + 1) * 512],
                                               start=(c == 0), stop=(c == 7))
                  stg = ls_p.tile([65, 2048], F32, tag="stg", name="stg")
                  for qt in range(4):
                      nc.vector.tensor_copy(stg[:, qt * 512:(qt + 1) * 512], oaccs[qt][0:65, :])
                  dpk = lpk_p.tile([128, 16], F32, tag="dpk", name="dpk")
                  nc.sync.dma_start(dpk[:], stg[64:65, :].rearrange("o (p f) -> o p f", f=16))
                  rpk = lpk_p.tile([128, 16], F32, tag="rpk", name="rpk")
                  nc.vector.reciprocal_approx_fast(rpk[:], dpk[:])
                  rrow = lpk_p.tile([1, 2048], F32, tag="rrow", name="rrow")
                  nc.sync.dma_start(rrow[:].rearrange("o (p f) -> o p f", f=16), rpk[:])
                  rb = lrb_p.tile([64, 2048], F32, tag="rb", name="rb")
                  nc.gpsimd.partition_broadcast(rb[:], rrow[:])
                  nc.vector.tensor_tensor(norm_l[h][:, qb:qb + 2048],
                                          stg[0:64, :], rb[:], mybir.AluOpType.mult)

    # =================== lofi projection -> y[256:512] ===================
    if "L" in PH:
      with tc.tile_pool(name="lpps", bufs=4, space="PSUM") as pps, \
           tc.tile_pool(name="lpyb", bufs=4) as yb_p:
          for mt in range(2):
              for nt in range(8):
                  ps = pps.tile([128, 512], F32, tag="ps", name="ps")
                  for h in range(4):
                      nc.tensor.matmul(ps[:],
                                       wlp[:, h * 256 + mt * 128: h * 256 + (mt + 1) * 128],
                                       norm_l[h][:, nt * 512:(nt + 1) * 512],
                                       start=(h == 0), stop=(h == 3))
                  yb = yb_p.tile([128, 512], F32, tag="yb", name="yb")
                  nc.scalar.activation(yb[:], ps[:], AF.Identity, bias=blp[:, mt:mt + 1], scale=1.0)
                  nc.sync.dma_start(y_d[256 + mt * 128: 256 + (mt + 1) * 128,
                                        nt * 512:(nt + 1) * 512], yb[:])

    nbl.release()
    opl.release()
    wp.release()


def _prep_weights(W_hqkv, b_hqkv, W_hproj, b_hproj, W_lq, b_lq, W_lkv, b_lkv,
                  W_lproj, b_lproj):
    f = np.float32
    bf = ml_dtypes.bfloat16
    wqk = np.ascontiguousarray(np.asarray(W_hqkv)[:512].T, dtype=bf)
    bqk = np.ascontiguousarray(np.asarray(b_hqkv)[:512].reshape(4, 128).T, dtype=f)
    whv = np.zeros((512, 260), bf)
    bhv = np.zeros((1, 260), bf)
    for h in range(4):
        whv[:, 65 * h:65 * h + 64] = np.asarray(W_hqkv)[512 + 64 * h:512 + 64 * (h + 1)].T
        bhv[0, 65 * h:65 * h + 64] = np.asarray(b_hqkv)[512 + 64 * h:512 + 64 * (h + 1)]
        bhv[0, 65 * h + 64] = 1.0
    wlq = np.ascontiguousarray(np.asarray(W_lq).T, dtype=bf)
    blq = np.ascontiguousarray(np.asarray(b_lq).reshape(2, 128).T, dtype=f)
    wlk = np.ascontiguousarray((0.25 * np.asarray(W_lkv)[:256]).T, dtype=bf)
    blk = np.ascontiguousarray(np.asarray(b_lkv)[:256].reshape(2, 128).T, dtype=f)
    wlv = np.zeros((512, 260), bf)
    blv = np.zeros((1, 260), bf)
    for h in range(4):
        wlv[:, 65 * h:65 * h + 64] = 0.25 * np.asarray(W_lkv)[256 + 64 * h:256 + 64 * (h + 1)].T
        blv[0, 65 * h:65 * h + 64] = np.asarray(b_lkv)[256 + 64 * h:256 + 64 * (h + 1)]
        blv[0, 65 * h + 64] = 1.0
    # proj weights: bf16, transposed (c, o); contraction per-head (64-row slices)
    whp = np.ascontiguousarray(
        np.asarray(W_hproj).T.reshape(4, 64, 256).transpose(1, 0, 2).reshape(64, 1024), dtype=bf)
    bhp = np.ascontiguousarray(np.asarray(b_hproj).reshape(2, 128).T, dtype=f)
    wlp = np.ascontiguousarray(
        np.asarray(W_lproj).T.reshape(4, 64, 256).transpose(1, 0, 2).reshape(64, 1024), dtype=bf)
    blp = np.ascontiguousarray(np.asarray(b_lproj).reshape(2, 128).T, dtype=f)
    mskL = np.zeros((128, 128), bf)
    mskR = np.zeros((128, 128), bf)
    for half in (0, 64):
        for g in range(32):
            mskL[half + g, 4 * g:4 * g + 4] = 1.0
            mskR[half + g, 4 * g:4 * g + 4] = CBIG
        mskL[half + 32, :] = 1.0
        mskR[half + 32, :] = -CBIG
    return dict(wqk=wqk, bqk=bqk, whv=whv, bhv=bhv, wlq=wlq, blq=blq,
                wlk=wlk, blk=blk, wlv=wlv, blv=blv, whp=whp, bhp=bhp,
                wlp=wlp, blp=blp, mskL=mskL, mskR=mskR)


def kernel(x, W_hqkv, b_hqkv, W_hproj, b_hproj, W_lq, b_lq, W_lkv, b_lkv,
           W_lproj, b_lproj, _trace=False):
    if "nc" not in _CACHE:
        _CACHE["nc"] = _build_bass()
    nc = _CACHE["nc"]
    wmap = _prep_weights(W_hqkv, b_hqkv, W_hproj, b_hproj, W_lq, b_lq,
                         W_lkv, b_lkv, W_lproj, b_lproj)
    x = np.asarray(x)
    B = x.shape[0]
    in_maps = []
    for b in range(8):
        m = dict(wmap)
        xi = np.ascontiguousarray(x[b % B].reshape(512, N), dtype=np.float32)
        m["x"] = xi.astype(ml_dtypes.bfloat16)
        m["xf"] = xi
        in_maps.append(m)
    res = run_bass_kernel_spmd(nc, in_maps, core_ids=list(range(8)), trace=_trace)
    _CACHE["last_res"] = res
    y = np.stack([res.results[b]["y"].reshape(512, 64, 64) for b in range(B)])
    return y



# revision 3
# speedup vs baseline: 1.1692x; 1.1692x over previous
"""Trainium2 Bass kernel for dual-branch (hifi windowed + lofi downsampled-KV)
attention. Data-parallel over batch: 8 batches -> 8 NeuronCores.

v2: head-PAIR packing throughout.
  - 64-contraction logit matmuls issued as row-group pairs (rows 0-63 and
    64-127 via tile_position auto-derive) -> both heads of a pair run
    concurrently on the PE array.
  - per-head normalized outputs packed into [128, N] pair tiles -> the output
    projections contract over 128 rows (2 accumulating matmuls instead of 4).
  - window mask folded into the logits PSUM with N=512 wide mask matmuls.
  - pooling runs in bf16 (no separate f32 input copy).
  - projection bias-adds on DVE (tensor_scalar) so ACT is ~exp-only; exp is
    the per-core floor (~19M elements through the ACT LUT).
  - softmax denominators: ones-column in the V^T aug weights -> row 64 of the
    attn@V PSUM; packed via small DMA transpose -> 128-lane reciprocal ->
    DMA back -> gpsimd partition_broadcast -> DVE normalize (fused with the
    hifi (g,i)->(h,w) output scatter).
"""
import sys

sys.path.insert(0, "/opt/trn_rl_repo")

import numpy as np
import ml_dtypes

import concourse.bass as bass
import concourse.bacc as bacc
import concourse.mybir as mybir
import concourse.tile as tile
from concourse.bass_utils import run_bass_kernel_spmd

F32 = mybir.dt.float32
BF16 = mybir.dt.bfloat16
AF = mybir.ActivationFunctionType
MUL = mybir.AluOpType.mult

SCALE = 64 ** -0.5   # 0.125
N = 4096
M = 1024
CBIG = 320.0         # mask magnitude pre-scale (C/SCALE with C=40)

_CACHE = {}


def _build_bass():
    nc = bacc.Bacc("TRN2", target_bir_lowering=False, debug=False, num_devices=8)

    d = {}
    d["x_d"] = nc.dram_tensor("x", (512, N), BF16, kind="ExternalInput").ap()
    for nm, shp, dt in [
        ("wqk", (512, 512), BF16), ("bqk", (128, 4), F32),
        ("whv", (512, 260), BF16), ("bhv", (1, 260), BF16),
        ("wlq", (512, 256), BF16), ("blq", (128, 2), F32),
        ("wlk", (512, 256), BF16), ("blk", (128, 2), F32),
        ("wlv", (512, 260), BF16), ("blv", (1, 260), BF16),
        ("whp", (128, 512), BF16), ("bhp", (128, 2), F32),
        ("wlp", (128, 512), BF16), ("blp", (128, 2), F32),
        ("mskL", (128, 128), BF16), ("mskR", (128, 512), BF16),
    ]:
        d[nm + "_d"] = nc.dram_tensor(nm, shp, dt, kind="ExternalInput").ap()
    d["y_d"] = nc.dram_tensor("y", (512, N), F32, kind="ExternalOutput").ap()

    with tile.TileContext(nc) as tc:
        _emit(nc, tc, d)
    nc.finalize()
    return nc


def _emit(nc, tc, d):
    x_d = d["x_d"]; y_d = d["y_d"]

    # ---- persistent: weights ----
    wp = tc.alloc_tile_pool(name="wp", bufs=1)
    wqk = wp.tile([128, 512 * 4], BF16, tag="wqk", name="wqk")
    bqk = wp.tile([128, 4], F32, tag="bqk", name="bqk")
    whv = wp.tile([128, 260 * 4], BF16, tag="whv", name="whv")
    bhv = wp.tile([1, 260], BF16, tag="bhv", name="bhv")
    wlq = wp.tile([128, 256 * 4], BF16, tag="wlq", name="wlq")
    blq = wp.tile([128, 2], F32, tag="blq", name="blq")
    wlk = wp.tile([128, 256 * 4], BF16, tag="wlk", name="wlk")
    blk = wp.tile([128, 2], F32, tag="blk", name="blk")
    wlv = wp.tile([128, 260 * 4], BF16, tag="wlv", name="wlv")
    blv = wp.tile([1, 260], BF16, tag="blv", name="blv")
    whp = wp.tile([128, 512], BF16, tag="whp", name="whp")
    bhp = wp.tile([128, 2], F32, tag="bhp", name="bhp")
    wlp = wp.tile([128, 512], BF16, tag="wlp", name="wlp")
    blp = wp.tile([128, 2], F32, tag="blp", name="blp")
    mskL = wp.tile([128, 128], BF16, tag="mskL", name="mskL")
    mskR = wp.tile([128, 512], BF16, tag="mskR", name="mskR")
    ones1 = wp.tile([1, 128], BF16, tag="ones1", name="ones1")

    for (t, nm) in [(wqk, "wqk"), (whv, "whv"), (wlq, "wlq"), (wlk, "wlk"), (wlv, "wlv")]:
        dr = d[nm + "_d"]
        w = dr.shape[1]
        for kt in range(4):
            nc.sync.dma_start(t[:, kt * w:(kt + 1) * w], dr[kt * 128:(kt + 1) * 128, :])
    for (t, nm) in [(whp, "whp"), (wlp, "wlp"), (bqk, "bqk"), (bhv, "bhv"),
                    (blq, "blq"), (blk, "blk"), (blv, "blv"), (bhp, "bhp"),
                    (blp, "blp"), (mskL, "mskL"), (mskR, "mskR")]:
        nc.sync.dma_start(t[:], d[nm + "_d"][:, :])
    nc.vector.memset(ones1[:], 1.0)

    # ---- persistent pools, ordered by release time (LIFO stack) ----
    opl = tc.alloc_tile_pool(name="oplofi", bufs=1)       # lives until after lofi
    lq = [opl.tile([128, N], BF16, tag=f"lq{p}", name=f"lq{p}") for p in range(2)]
    lk = [opl.tile([128, M], BF16, tag=f"lk{p}", name=f"lk{p}") for p in range(2)]
    val = opl.tile([128, 260 * 8], BF16, tag="val", name="val")
    oph = tc.alloc_tile_pool(name="ophifi", bufs=1)       # lives until after hifi proj
    qkh = [oph.tile([128, N], BF16, tag=f"qkh{p}", name=f"qkh{p}") for p in range(4)]
    vah = oph.tile([128, 260 * 32], BF16, tag="vah", name="vah")
    opx = tc.alloc_tile_pool(name="opx", bufs=1)          # lives through P1 only
    xpb = [opx.tile([128, M], BF16, tag=f"xpb{kt}", name=f"xpb{kt}") for kt in range(4)]

    # =================== P1: projections ===================
    with tc.tile_pool(name="p1x", bufs=3) as xin_p, \
         tc.tile_pool(name="p1t1", bufs=3) as t1_p, \
         tc.tile_pool(name="p1ps", bufs=4, space="PSUM") as ps_p, \
         tc.tile_pool(name="p1ps2", bufs=4, space="PSUM") as ps2_p:
        for nt in range(8):
            xb = [xin_p.tile([128, 512], BF16, tag=f"xb{kt}", name=f"xb{kt}") for kt in range(4)]
            for kt in range(4):
                nc.sync.dma_start(xb[kt][:], x_d[kt * 128:(kt + 1) * 128, nt * 512:(nt + 1) * 512])
            # pooling (8 h-rows per block): w-pairs then h-pairs, bf16 (the /4
            # is folded into wlk/wlv)
            for kt in range(4):
                v = xb[kt][:].rearrange("p (h w2 two) -> p h w2 two", w2=32, two=2)
                t1 = t1_p.tile([128, 256], BF16, tag="t1", name="t1")
                t1v = t1[:].rearrange("p (h w2) -> p h w2", w2=32)
                nc.gpsimd.tensor_add(t1v, v[:, :, :, 0], v[:, :, :, 1])
                t1p = t1[:].rearrange("p (i two w2) -> p i two w2", two=2, w2=32)
                xpv = xpb[kt][:, nt * 128:(nt + 1) * 128].rearrange("p (i w2) -> p i w2", w2=32)
                nc.gpsimd.tensor_add(xpv, t1p[:, :, 0, :], t1p[:, :, 1, :])
            # hifi q,k
            for mt in range(4):
                ps = ps_p.tile([128, 512], F32, tag="ps", name="ps")
                for kt in range(4):
                    nc.tensor.matmul(ps[:], wqk[:, kt * 512 + mt * 128: kt * 512 + (mt + 1) * 128],
                                     xb[kt][:], start=(kt == 0), stop=(kt == 3))
                nc.scalar.activation(qkh[mt][:, nt * 512:(nt + 1) * 512], ps[:],
                                     AF.Identity, bias=bqk[:, mt:mt + 1], scale=1.0)
            # hifi V^T aug
            for sc in range(4):
                st = nt * 4 + sc
                ps = ps2_p.tile([128, 260], F32, tag="psv", name="psv")
                for kt in range(4):
                    nc.tensor.matmul(ps[:], xb[kt][:, sc * 128:(sc + 1) * 128],
                                     whv[:, kt * 260:(kt + 1) * 260], start=(kt == 0), stop=False)
                nc.tensor.matmul(ps[:], ones1[:], bhv[:], start=False, stop=True)
                nc.vector.tensor_copy(vah[:, st * 260:(st + 1) * 260], ps[:])
            # lofi q
            for mt in range(2):
                ps = ps_p.tile([128, 512], F32, tag="ps", name="ps")
                for kt in range(4):
                    nc.tensor.matmul(ps[:], wlq[:, kt * 256 + mt * 128: kt * 256 + (mt + 1) * 128],
                                     xb[kt][:], start=(kt == 0), stop=(kt == 3))
                nc.scalar.activation(lq[mt][:, nt * 512:(nt + 1) * 512], ps[:],
                                     AF.Identity, bias=blq[:, mt:mt + 1], scale=1.0)
        # lofi k
        for mt in range(2):
            for ntk in range(2):
                ps = ps_p.tile([128, 512], F32, tag="ps", name="ps")
                for kt in range(4):
                    nc.tensor.matmul(ps[:], wlk[:, kt * 256 + mt * 128: kt * 256 + (mt + 1) * 128],
                                     xpb[kt][:, ntk * 512:(ntk + 1) * 512], start=(kt == 0), stop=(kt == 3))
                nc.scalar.activation(lk[mt][:, ntk * 512:(ntk + 1) * 512], ps[:],
                                     AF.Identity, bias=blk[:, mt:mt + 1], scale=1.0)
        # lofi V^T aug
        for mc in range(8):
            ps = ps2_p.tile([128, 260], F32, tag="psv", name="psv")
            for kt in range(4):
                nc.tensor.matmul(ps[:], xpb[kt][:, mc * 128:(mc + 1) * 128],
                                 wlv[:, kt * 260:(kt + 1) * 260], start=(kt == 0), stop=False)
            nc.tensor.matmul(ps[:], ones1[:], blv[:], start=False, stop=True)
            nc.vector.tensor_copy(val[:, mc * 260:(mc + 1) * 260], ps[:])

    opx.release()

    # ---- pair-packed normalized hifi outputs (rows 0-63: even head,
    #      rows 64-127: odd head) ----
    nbh = tc.alloc_tile_pool(name="normbufh", bufs=1)
    norm_h = [nbh.tile([128, N], BF16, tag=f"nh{p}", name=f"nh{p}") for p in range(2)]

    # =================== hifi attention (head pairs) ===================
    with tc.tile_pool(name="hL", bufs=1, space="PSUM") as hL_p, \
         tc.tile_pool(name="hO", bufs=1, space="PSUM") as hO_p, \
         tc.tile_pool(name="hexp", bufs=3) as he_p, \
         tc.tile_pool(name="hstg", bufs=3) as hs_p, \
         tc.tile_pool(name="hpk", bufs=4) as hpk_p, \
         tc.tile_pool(name="hrb", bufs=3) as hrb_p:
        for p in range(2):
            qt_ = qkh[p]
            kt_ = qkh[2 + p]
            for sg in range(4):
                Lg = [hL_p.tile([128, 1024], F32, tag=f"Lg{i}", name=f"Lg{i}") for i in range(2)]
                # paired logits: head 2p on rows 0-63, head 2p+1 on rows 64-127;
                # the window-mask bias matmul must target the exact same PSUM
                # region as its data matmul (one accumulation group) so the
                # scheduler cannot reorder mask before data.
                for s4 in range(8):
                    st = sg * 8 + s4
                    for i in range(2):
                        r = 64 * i
                        nc.tensor.matmul(Lg[i][:, s4 * 128:(s4 + 1) * 128],
                                         kt_[r:r + 64, st * 128:(st + 1) * 128],
                                         qt_[r:r + 64, st * 128:(st + 1) * 128],
                                         start=True, stop=False)
                        nc.tensor.matmul(Lg[i][:, s4 * 128:(s4 + 1) * 128],
                                         mskL[r:r + 64, :], mskR[r:r + 64, 0:128],
                                         start=False, stop=True)
                eL = [he_p.tile([128, 1024], BF16, tag=f"eL{i}", name=f"eL{i}") for i in range(2)]
                for i in range(2):
                    nc.scalar.activation(eL[i][:], Lg[i][:], AF.Exp, scale=SCALE)
                oh = [hO_p.tile([65, 1024], F32, tag=f"oh{i}", name=f"oh{i}") for i in range(2)]
                for s4 in range(8):
                    st = sg * 8 + s4
                    for i in range(2):
                        h = 2 * p + i
                        nc.tensor.matmul(oh[i][:, s4 * 128:(s4 + 1) * 128],
                                         vah[:, st * 260 + h * 65: st * 260 + (h + 1) * 65],
                                         eL[i][:, s4 * 128:(s4 + 1) * 128], start=True, stop=True)
                stg = [hs_p.tile([65, 1024], F32, tag=f"stg{i}", name=f"stg{i}") for i in range(2)]
                for i in range(2):
                    nc.vector.tensor_copy(stg[i][:], oh[i][:])
                # denominators: pack both heads into one 128-partition tile
                dpk = hpk_p.tile([128, 16], F32, tag="dpk", name="dpk")
                for i in range(2):
                    nc.sync.dma_start(dpk[:, 8 * i:8 * i + 8],
                                      stg[i][64:65, :].rearrange("o (p f) -> o p f", f=8))
                rpk = hpk_p.tile([128, 16], F32, tag="rpk", name="rpk")
                nc.vector.reciprocal_approx_fast(rpk[:], dpk[:])
                for i in range(2):
                    rrow = hpk_p.tile([1, 1024], F32, tag=f"rrow{i}", name=f"rrow{i}")
                    nc.sync.dma_start(rrow[:].rearrange("o (p f) -> o p f", f=8),
                                      rpk[:, 8 * i:8 * i + 8])
                    rb = hrb_p.tile([64, 1024], F32, tag=f"rb{i}", name=f"rb{i}")
                    nc.gpsimd.partition_broadcast(rb[:], rrow[:])
                    # normalize + scatter: src col (s8,g,i1,i2) -> dst 128*s8+64*i1+2*g+i2
                    dstv = norm_h[p][64 * i:64 * i + 64, sg * 1024:(sg + 1) * 1024].rearrange(
                        "p (s i1 gi2) -> p s i1 gi2", s=8, i1=2, gi2=64)
                    srcv = stg[i][0:64, :].rearrange("p (s g i1 i2) -> p s g i1 i2", s=8, g=32, i1=2, i2=2)
                    rbv = rb[:].rearrange("p (s g i1 i2) -> p s g i1 i2", s=8, g=32, i1=2, i2=2)
                    for i1 in range(2):
                        nc.vector.tensor_tensor(dstv[:, :, i1, :], srcv[:, :, :, i1, :],
                                                rbv[:, :, :, i1, :], MUL)

    # =================== hifi projection -> y[0:256] ===================
    with tc.tile_pool(name="hpps", bufs=4, space="PSUM") as pps, \
         tc.tile_pool(name="hpyb", bufs=4) as yb_p:
        for mt in range(2):
            for nt in range(8):
                ps = pps.tile([128, 512], F32, tag="ps", name="ps")
                for p in range(2):
                    nc.tensor.matmul(ps[:],
                                     whp[:, p * 256 + mt * 128: p * 256 + (mt + 1) * 128],
                                     norm_h[p][:, nt * 512:(nt + 1) * 512],
                                     start=(p == 0), stop=(p == 1))
                yb = yb_p.tile([128, 512], F32, tag="yb", name="yb")
                nc.vector.tensor_scalar_add(yb[:], ps[:], bhp[:, mt:mt + 1])
                nc.sync.dma_start(y_d[mt * 128:(mt + 1) * 128, nt * 512:(nt + 1) * 512], yb[:])

    nbh.release()
    oph.release()

    # ---- pair-packed normalized lofi outputs ----
    nbl = tc.alloc_tile_pool(name="normbufl", bufs=1)
    norm_l = [nbl.tile([128, N], BF16, tag=f"nl{p}", name=f"nl{p}") for p in range(2)]

    # =================== lofi attention (head pairs) ===================
    with tc.tile_pool(name="lS", bufs=1, space="PSUM") as lS_p, \
         tc.tile_pool(name="lO", bufs=1, space="PSUM") as lO_p, \
         tc.tile_pool(name="lexp", bufs=3) as le_p, \
         tc.tile_pool(name="lstg", bufs=3) as ls_p, \
         tc.tile_pool(name="lpk", bufs=4) as lpk_p, \
         tc.tile_pool(name="lrb", bufs=3) as lrb_p:
        for p in range(2):
            for qb in range(4):
                q0 = qb * 1024
                oacc = [lO_p.tile([65, 1024], F32, tag=f"oac{i}", name=f"oac{i}") for i in range(2)]
                for c in range(8):
                    sg = [lS_p.tile([128, 1024], F32, tag=f"sg{i}", name=f"sg{i}") for i in range(2)]
                    # paired S^T: head 2p rows 0-63, head 2p+1 rows 64-127
                    for q2 in range(2):
                        for i in range(2):
                            r = 64 * i
                            nc.tensor.matmul(sg[i][:, q2 * 512:(q2 + 1) * 512],
                                             lk[p][r:r + 64, c * 128:(c + 1) * 128],
                                             lq[p][r:r + 64, q0 + q2 * 512: q0 + (q2 + 1) * 512],
                                             start=True, stop=True)
                    eS = [le_p.tile([128, 1024], BF16, tag=f"eS{i}", name=f"eS{i}") for i in range(2)]
                    for i in range(2):
                        nc.scalar.activation(eS[i][:], sg[i][:], AF.Exp, scale=SCALE)
                    for q2 in range(2):
                        for i in range(2):
                            h = 2 * p + i
                            nc.tensor.matmul(oacc[i][:, q2 * 512:(q2 + 1) * 512],
                                             val[:, c * 260 + h * 65: c * 260 + (h + 1) * 65],
                                             eS[i][:, q2 * 512:(q2 + 1) * 512],
                                             start=(c == 0), stop=(c == 7))
                stg = [ls_p.tile([65, 1024], F32, tag=f"stg{i}", name=f"stg{i}") for i in range(2)]
                for i in range(2):
                    nc.vector.tensor_copy(stg[i][:], oacc[i][:])
                dpk = lpk_p.tile([128, 16], F32, tag="dpk", name="dpk")
                for i in range(2):
                    nc.sync.dma_start(dpk[:, 8 * i:8 * i + 8],
                                      stg[i][64:65, :].rearrange("o (p f) -> o p f", f=8))
                rpk = lpk_p.tile([128, 16], F32, tag="rpk", name="rpk")
                nc.vector.reciprocal_approx_fast(rpk[:], dpk[:])
                for i in range(2):
                    rrow = lpk_p.tile([1, 1024], F32, tag=f"rrow{i}", name=f"rrow{i}")
                    nc.sync.dma_start(rrow[:].rearrange("o (p f) -> o p f", f=8),
                                      rpk[:, 8 * i:8 * i + 8])
                    rb = lrb_p.tile([64, 1024], F32, tag=f"rb{i}", name=f"rb{i}")
                    nc.gpsimd.partition_broadcast(rb[:], rrow[:])
                    nc.vector.tensor_tensor(norm_l[p][64 * i:64 * i + 64, q0:q0 + 1024],
                                            stg[i][0:64, :], rb[:], MUL)

    # =================== lofi projection -> y[256:512] ===================
    with tc.tile_pool(name="lpps", bufs=4, space="PSUM") as pps, \
         tc.tile_pool(name="lpyb", bufs=4) as yb_p:
        for mt in range(2):
            for nt in range(8):
                ps = pps.tile([128, 512], F32, tag="ps", name="ps")
                for p in range(2):
                    nc.tensor.matmul(ps[:],
                                     wlp[:, p * 256 + mt * 128: p * 256 + (mt + 1) * 128],
                                     norm_l[p][:, nt * 512:(nt + 1) * 512],
                                     start=(p == 0), stop=(p == 1))
                yb = yb_p.tile([128, 512], F32, tag="yb", name="yb")
                nc.vector.tensor_scalar_add(yb[:], ps[:], blp[:, mt:mt + 1])
                nc.sync.dma_start(y_d[256 + mt * 128: 256 + (mt + 1) * 128,
                                      nt * 512:(nt + 1) * 512], yb[:])

    nbl.release()
    opl.release()
    wp.release()


def _prep_weights(W_hqkv, b_hqkv, W_hproj, b_hproj, W_lq, b_lq, W_lkv, b_lkv,
                  W_lproj, b_lproj):
    f = np.float32
    bf = ml_dtypes.bfloat16
    wqk = np.ascontiguousarray(np.asarray(W_hqkv)[:512].T, dtype=bf)
    bqk = np.ascontiguousarray(np.asarray(b_hqkv)[:512].reshape(4, 128).T, dtype=f)
    whv = np.zeros((512, 260), bf)
    bhv = np.zeros((1, 260), bf)
    for h in range(4):
        whv[:, 65 * h:65 * h + 64] = np.asarray(W_hqkv)[512 + 64 * h:512 + 64 * (h + 1)].T
        bhv[0, 65 * h:65 * h + 64] = np.asarray(b_hqkv)[512 + 64 * h:512 + 64 * (h + 1)]
        bhv[0, 65 * h + 64] = 1.0
    wlq = np.ascontiguousarray(np.asarray(W_lq).T, dtype=bf)
    blq = np.ascontiguousarray(np.asarray(b_lq).reshape(2, 128).T, dtype=f)
    wlk = np.ascontiguousarray((0.25 * np.asarray(W_lkv)[:256]).T, dtype=bf)
    blk = np.ascontiguousarray(np.asarray(b_lkv)[:256].reshape(2, 128).T, dtype=f)
    wlv = np.zeros((512, 260), bf)
    blv = np.zeros((1, 260), bf)
    for h in range(4):
        wlv[:, 65 * h:65 * h + 64] = 0.25 * np.asarray(W_lkv)[256 + 64 * h:256 + 64 * (h + 1)].T
        blv[0, 65 * h:65 * h + 64] = np.asarray(b_lkv)[256 + 64 * h:256 + 64 * (h + 1)]
        blv[0, 65 * h + 64] = 1.0
    # proj weights: bf16, transposed (in, out), pair-packed: rows 0-127 are the
    # pair's input channels; cols [p*256 + mt*128 ...] select (pair, out tile)
    whp = np.ascontiguousarray(
        np.asarray(W_hproj).T.reshape(2, 128, 256).transpose(1, 0, 2).reshape(128, 512), dtype=bf)
    bhp = np.ascontiguousarray(np.asarray(b_hproj).reshape(2, 128).T, dtype=f)
    wlp = np.ascontiguousarray(
        np.asarray(W_lproj).T.reshape(2, 128, 256).transpose(1, 0, 2).reshape(128, 512), dtype=bf)
    blp = np.ascontiguousarray(np.asarray(b_lproj).reshape(2, 128).T, dtype=f)
    mskL = np.zeros((128, 128), bf)
    mskR = np.zeros((128, 512), bf)
    for half in (0, 64):
        for g in range(32):
            mskL[half + g, 4 * g:4 * g + 4] = 1.0
            for t in range(4):
                mskR[half + g, 128 * t + 4 * g:128 * t + 4 * g + 4] = CBIG
        mskL[half + 32, :] = 1.0
        mskR[half + 32, :] = -CBIG
    return dict(wqk=wqk, bqk=bqk, whv=whv, bhv=bhv, wlq=wlq, blq=blq,
                wlk=wlk, blk=blk, wlv=wlv, blv=blv, whp=whp, bhp=bhp,
                wlp=wlp, blp=blp, mskL=mskL, mskR=mskR)


def kernel(x, W_hqkv, b_hqkv, W_hproj, b_hproj, W_lq, b_lq, W_lkv, b_lkv,
           W_lproj, b_lproj, _trace=False):
    if "nc" not in _CACHE:
        _CACHE["nc"] = _build_bass()
    nc = _CACHE["nc"]
    wmap = _prep_weights(W_hqkv, b_hqkv, W_hproj, b_hproj, W_lq, b_lq,
                         W_lkv, b_lkv, W_lproj, b_lproj)
    x = np.asarray(x)
    B = x.shape[0]
    in_maps = []
    for b in range(8):
        m = dict(wmap)
        m["x"] = np.ascontiguousarray(x[b % B].reshape(512, N), dtype=ml_dtypes.bfloat16)
        in_maps.append(m)
    res = run_bass_kernel_spmd(nc, in_maps, core_ids=list(range(8)), trace=_trace)
    _CACHE["last_res"] = res
    y = np.stack([res.results[b]["y"].reshape(512, 64, 64) for b in range(B)])
    return y


# revision 4
# speedup vs baseline: 1.3029x; 1.1143x over previous
"""Trainium2 Bass kernel for dual-branch (hifi windowed + lofi downsampled-KV)
attention. Data-parallel over batch: 8 batches -> 8 NeuronCores.

v4: the per-core floor is the ACT (ScalarE) exp stream (~19M softmax elements
at 1 elem/lane/cycle), so the kernel is structured to keep ACT saturated from
~20us onward and hide everything else under it:

  Phase 0: DMA all of x into SBUF; 2x2 average-pool in bf16 on DVE; project
           lofi k and V^T (keys on partitions, ones-column for denominators).
  Phase 1 (per 512-pixel tile nt): project lofi q, then run the two lofi
           attention blocks for these 512 queries (head PAIRS packed: S^T via
           concurrent row-group matmuls into one [128,1024] PSUM tile holding
           [h_even 512q | h_odd 512q]; ONE exp per chunk; attn@V lag-1 behind
           exp so the PE never stalls on ACT), with the hifi q/k/V^T
           projection matmuls sprinkled between chunks as PE filler.
  Phase 2: hifi windowed attention (pair-packed logits+mask into [128,1024]
           PSUM, one exp per 512-pixel block), with the lofi output
           projection interleaved as PE filler.
  Phase 3: hifi output projection.

  Softmax denominators ride as a ones-column in the V^T weights -> row 64 of
  the attn@V output; packed via DMA transpose -> 128-lane reciprocal -> DMA
  back -> gpsimd partition_broadcast -> DVE normalize (fused with the hifi
  (g,i)->(h,w) scatter), writing pair-packed [128,N] tiles so the output
  projections contract over the full 128 partitions.
"""
import sys

sys.path.insert(0, "/opt/trn_rl_repo")

import numpy as np
import ml_dtypes

import concourse.bass as bass
import concourse.bacc as bacc
import concourse.mybir as mybir
import concourse.tile as tile
from concourse.bass_utils import run_bass_kernel_spmd

F32 = mybir.dt.float32
BF16 = mybir.dt.bfloat16
AF = mybir.ActivationFunctionType
MUL = mybir.AluOpType.mult

SCALE = 64 ** -0.5   # 0.125
N = 4096
M = 1024
CBIG = 320.0         # mask magnitude pre-scale (C/SCALE with C=40)

_CACHE = {}


def _build_bass():
    nc = bacc.Bacc("TRN2", target_bir_lowering=False, debug=False, num_devices=8)

    d = {}
    d["x_d"] = nc.dram_tensor("x", (512, N), BF16, kind="ExternalInput").ap()
    for nm, shp, dt in [
        ("wqk", (512, 512), BF16), ("bqk", (128, 4), F32),
        ("whv", (512, 260), BF16), ("bhv", (1, 260), BF16),
        ("wlq", (512, 256), BF16), ("blq", (128, 2), F32),
        ("wlk", (512, 256), BF16), ("blk", (128, 2), F32),
        ("wlv", (512, 260), BF16), ("blv", (1, 260), BF16),
        ("whp", (128, 512), BF16), ("bhp", (128, 2), F32),
        ("wlp", (128, 512), BF16), ("blp", (128, 2), F32),
        ("mskL", (128, 128), BF16), ("mskR", (128, 512), BF16),
    ]:
        d[nm + "_d"] = nc.dram_tensor(nm, shp, dt, kind="ExternalInput").ap()
    d["y_d"] = nc.dram_tensor("y", (512, N), F32, kind="ExternalOutput").ap()

    with tile.TileContext(nc) as tc:
        _emit(nc, tc, d)
    nc.finalize()
    return nc


def _emit(nc, tc, d):
    x_d = d["x_d"]; y_d = d["y_d"]

    # ---- persistent: weights ----
    wp = tc.alloc_tile_pool(name="wp", bufs=1)
    wqk = wp.tile([128, 512 * 4], BF16, tag="wqk", name="wqk")
    bqk = wp.tile([128, 4], F32, tag="bqk", name="bqk")
    whv = wp.tile([128, 260 * 4], BF16, tag="whv", name="whv")
    bhv = wp.tile([1, 260], BF16, tag="bhv", name="bhv")
    wlq = wp.tile([128, 256 * 4], BF16, tag="wlq", name="wlq")
    blq = wp.tile([128, 2], F32, tag="blq", name="blq")
    wlk = wp.tile([128, 256 * 4], BF16, tag="wlk", name="wlk")
    blk = wp.tile([128, 2], F32, tag="blk", name="blk")
    wlv = wp.tile([128, 260 * 4], BF16, tag="wlv", name="wlv")
    blv = wp.tile([1, 260], BF16, tag="blv", name="blv")
    whp = wp.tile([128, 512], BF16, tag="whp", name="whp")
    bhp = wp.tile([128, 2], F32, tag="bhp", name="bhp")
    wlp = wp.tile([128, 512], BF16, tag="wlp", name="wlp")
    blp = wp.tile([128, 2], F32, tag="blp", name="blp")
    mskL = wp.tile([128, 128], BF16, tag="mskL", name="mskL")
    mskR = wp.tile([128, 512], BF16, tag="mskR", name="mskR")
    ones1 = wp.tile([1, 128], BF16, tag="ones1", name="ones1")

    for (t, nm) in [(wqk, "wqk"), (whv, "whv"), (wlq, "wlq"), (wlk, "wlk"), (wlv, "wlv")]:
        dr = d[nm + "_d"]
        w = dr.shape[1]
        for kt in range(4):
            nc.sync.dma_start(t[:, kt * w:(kt + 1) * w], dr[kt * 128:(kt + 1) * 128, :])
    for (t, nm) in [(whp, "whp"), (wlp, "wlp"), (bqk, "bqk"), (bhv, "bhv"),
                    (blq, "blq"), (blk, "blk"), (blv, "blv"), (bhp, "bhp"),
                    (blp, "blp"), (mskL, "mskL"), (mskR, "mskR")]:
        nc.sync.dma_start(t[:], d[nm + "_d"][:, :])
    nc.vector.memset(ones1[:], 1.0)

    # ---- persistent pools, ordered by release time (LIFO stack) ----
    opl = tc.alloc_tile_pool(name="oplofi", bufs=1)       # lives until after lofi proj
    lq = [opl.tile([128, N], BF16, tag=f"lq{p}", name=f"lq{p}") for p in range(2)]
    lk = [opl.tile([128, M], BF16, tag=f"lk{p}", name=f"lk{p}") for p in range(2)]
    val = opl.tile([128, 260 * 8], BF16, tag="val", name="val")
    norm_l = [opl.tile([128, N], BF16, tag=f"nl{p}", name=f"nl{p}") for p in range(2)]
    oph = tc.alloc_tile_pool(name="ophifi", bufs=1)       # lives until after hifi proj
    qkh = [oph.tile([128, N], BF16, tag=f"qkh{p}", name=f"qkh{p}") for p in range(4)]
    vah = oph.tile([128, 260 * 32], BF16, tag="vah", name="vah")
    norm_h = [oph.tile([128, N], BF16, tag=f"nh{p}", name=f"nh{p}") for p in range(2)]
    opx = tc.alloc_tile_pool(name="opx", bufs=1)          # x tiles; through phase 1
    xb = [[opx.tile([128, 512], BF16, tag=f"xb{nt}_{kt}", name=f"xb{nt}_{kt}")
           for kt in range(4)] for nt in range(8)]
    xpb = [opx.tile([128, M], BF16, tag=f"xpb{kt}", name=f"xpb{kt}") for kt in range(4)]

    # =================== Phase 0: x load, pooling, lofi k/V ===================
    for nt in range(8):
        for kt in range(4):
            nc.sync.dma_start(xb[nt][kt][:], x_d[kt * 128:(kt + 1) * 128, nt * 512:(nt + 1) * 512])
    with tc.tile_pool(name="p0t1", bufs=3) as t1_p, \
         tc.tile_pool(name="p0ps", bufs=2, space="PSUM") as ps0_p:
        # 2x2 avg-pool, bf16 on DVE (the /4 is folded into wlk/wlv)
        for nt in range(8):
            for kt in range(4):
                v = xb[nt][kt][:].rearrange("p (h w2 two) -> p h w2 two", w2=32, two=2)
                t1 = t1_p.tile([128, 256], BF16, tag="t1", name="t1")
                t1v = t1[:].rearrange("p (h w2) -> p h w2", w2=32)
                nc.vector.tensor_add(t1v, v[:, :, :, 0], v[:, :, :, 1])
                t1p = t1[:].rearrange("p (i two w2) -> p i two w2", two=2, w2=32)
                xpv = xpb[kt][:, nt * 128:(nt + 1) * 128].rearrange("p (i w2) -> p i w2", w2=32)
                nc.vector.tensor_add(xpv, t1p[:, :, 0, :], t1p[:, :, 1, :])
        # lofi k
        for mt in range(2):
            for ntk in range(2):
                ps = ps0_p.tile([128, 512], F32, tag="ps", name="ps")
                for kt in range(4):
                    nc.tensor.matmul(ps[:], wlk[:, kt * 256 + mt * 128: kt * 256 + (mt + 1) * 128],
                                     xpb[kt][:, ntk * 512:(ntk + 1) * 512], start=(kt == 0), stop=(kt == 3))
                nc.vector.tensor_scalar_add(lk[mt][:, ntk * 512:(ntk + 1) * 512], ps[:],
                                            blk[:, mt:mt + 1])
        # lofi V^T aug
        for mc in range(8):
            ps = ps0_p.tile([128, 512], F32, tag="ps", name="ps")
            for kt in range(4):
                nc.tensor.matmul(ps[:, 0:260], xpb[kt][:, mc * 128:(mc + 1) * 128],
                                 wlv[:, kt * 260:(kt + 1) * 260], start=(kt == 0), stop=False)
            nc.tensor.matmul(ps[:, 0:260], ones1[:], blv[:], start=False, stop=True)
            nc.vector.tensor_copy(val[:, mc * 260:(mc + 1) * 260], ps[:, 0:260])

    # =================== Phase 1: lofi attention + hifi/lofi q projections ===
    # Per nt (512 pixels): lofi q for these queries, then the two head-pair
    # blocks (p=0, p=1) over all 1024 keys; hifi qk/V^T matmul groups
    # sprinkled between chunks as PE filler while ACT runs exp.
    with tc.tile_pool(name="p1ps", bufs=2, space="PSUM") as ps_p, \
         tc.tile_pool(name="lS", bufs=2, space="PSUM") as lS_p, \
         tc.tile_pool(name="lO", bufs=1, space="PSUM") as lO_p, \
         tc.tile_pool(name="lexp", bufs=3) as le_p, \
         tc.tile_pool(name="lstg", bufs=3) as ls_p, \
         tc.tile_pool(name="lpk", bufs=4) as lpk_p, \
         tc.tile_pool(name="lrb", bufs=3) as lrb_p:

        def p1_group(nt, g):
            # hifi q,k (g=0..3) and hifi V^T (g=4..7) for tile nt
            if g < 4:
                mt = g
                ps = ps_p.tile([128, 512], F32, tag="ps", name="ps")
                for kt in range(4):
                    nc.tensor.matmul(ps[:], wqk[:, kt * 512 + mt * 128: kt * 512 + (mt + 1) * 128],
                                     xb[nt][kt][:], start=(kt == 0), stop=(kt == 3))
                nc.vector.tensor_scalar_add(qkh[mt][:, nt * 512:(nt + 1) * 512], ps[:],
                                            bqk[:, mt:mt + 1])
            else:
                sc = g - 4
                st = nt * 4 + sc
                ps = ps_p.tile([128, 512], F32, tag="ps", name="ps")
                for kt in range(4):
                    nc.tensor.matmul(ps[:, 0:260], xb[nt][kt][:, sc * 128:(sc + 1) * 128],
                                     whv[:, kt * 260:(kt + 1) * 260], start=(kt == 0), stop=False)
                nc.tensor.matmul(ps[:, 0:260], ones1[:], bhv[:], start=False, stop=True)
                nc.vector.tensor_copy(vah[:, st * 260:(st + 1) * 260], ps[:, 0:260])

        for nt in range(8):
            q0 = nt * 512
            # lofi q for this pixel tile
            for mt in range(2):
                ps = ps_p.tile([128, 512], F32, tag="ps", name="ps")
                for kt in range(4):
                    nc.tensor.matmul(ps[:], wlq[:, kt * 256 + mt * 128: kt * 256 + (mt + 1) * 128],
                                     xb[nt][kt][:], start=(kt == 0), stop=(kt == 3))
                nc.vector.tensor_scalar_add(lq[mt][:, q0:q0 + 512], ps[:], blq[:, mt:mt + 1])

            for p in range(2):
                # one lofi block: pair p, queries q0..q0+512, all 1024 keys.
                # sg/eS hold [h_even 512q | h_odd 512q]; attn@V lags exp by one
                # chunk so the PE never waits on ACT.
                oacc = lO_p.tile([65, 1024], F32, tag="oac", name="oac")
                eSs = []
                for c in range(8):
                    sg = lS_p.tile([128, 1024], F32, tag="sg", name="sg")
                    for i in range(2):
                        r = 64 * i
                        nc.tensor.matmul(sg[:, i * 512:(i + 1) * 512],
                                         lk[p][r:r + 64, c * 128:(c + 1) * 128],
                                         lq[p][r:r + 64, q0:q0 + 512],
                                         start=True, stop=True)
                    eS = le_p.tile([128, 1024], BF16, tag="eS", name="eS")
                    nc.scalar.activation(eS[:], sg[:], AF.Exp, scale=SCALE)
                    eSs.append(eS)
                    if c > 0:
                        _lofi_av(nc, oacc, val, eSs[c - 1], p, c - 1)
                    # PE filler: 4 P1 groups sprinkled into each block
                    if c % 2 == 1:
                        p1_group(nt, p * 4 + c // 2)
                _lofi_av(nc, oacc, val, eSs[7], p, 7)
                # evacuate + normalize
                stg = ls_p.tile([65, 1024], F32, tag="stg", name="stg")
                nc.vector.tensor_copy(stg[:], oacc[:])
                dpk = lpk_p.tile([128, 8], F32, tag="dpk", name="dpk")
                nc.sync.dma_start(dpk[:], stg[64:65, :].rearrange("o (p f) -> o p f", f=8))
                rpk = lpk_p.tile([128, 8], F32, tag="rpk", name="rpk")
                nc.vector.reciprocal_approx_fast(rpk[:], dpk[:])
                rrow = lpk_p.tile([1, 1024], F32, tag="rrow", name="rrow")
                nc.sync.dma_start(rrow[:].rearrange("o (p f) -> o p f", f=8), rpk[:])
                rb = lrb_p.tile([64, 1024], F32, tag="rb", name="rb")
                nc.gpsimd.partition_broadcast(rb[:], rrow[:])
                for i in range(2):
                    nc.vector.tensor_tensor(norm_l[p][64 * i:64 * i + 64, q0:q0 + 512],
                                            stg[0:64, i * 512:(i + 1) * 512],
                                            rb[:, i * 512:(i + 1) * 512], MUL)

    opx.release()

    # =================== Phase 2: hifi attention + lofi projection ===========
    # 16 blocks of (pair, 512 pixels); lofi projection groups interleaved as
    # PE filler.
    with tc.tile_pool(name="hL", bufs=2, space="PSUM") as hL_p, \
         tc.tile_pool(name="hO", bufs=1, space="PSUM") as hO_p, \
         tc.tile_pool(name="lpps", bufs=2, space="PSUM") as lpps, \
         tc.tile_pool(name="hexp", bufs=3) as he_p, \
         tc.tile_pool(name="hstg", bufs=3) as hs_p, \
         tc.tile_pool(name="hpk", bufs=4) as hpk_p, \
         tc.tile_pool(name="hrb", bufs=3) as hrb_p, \
         tc.tile_pool(name="lpyb", bufs=4) as lyb_p:

        def lproj_group(k):
            mt, nt = k // 8, k % 8
            ps = lpps.tile([128, 512], F32, tag="ps", name="ps")
            for p in range(2):
                nc.tensor.matmul(ps[:],
                                 wlp[:, p * 256 + mt * 128: p * 256 + (mt + 1) * 128],
                                 norm_l[p][:, nt * 512:(nt + 1) * 512],
                                 start=(p == 0), stop=(p == 1))
            yb = lyb_p.tile([128, 512], F32, tag="yb", name="yb")
            nc.vector.tensor_scalar_add(yb[:], ps[:], blp[:, mt:mt + 1])
            nc.sync.dma_start(y_d[256 + mt * 128: 256 + (mt + 1) * 128,
                                  nt * 512:(nt + 1) * 512], yb[:])

        blk_i = 0
        for p in range(2):
            qt_ = qkh[p]
            kt_ = qkh[2 + p]
            for sgh in range(8):
                # logits+mask for 512 px, both heads packed:
                # Lg = [h_even: 4 x 128px | h_odd: 4 x 128px]
                Lg = hL_p.tile([128, 1024], F32, tag="Lg", name="Lg")
                for s4 in range(4):
                    st = sgh * 4 + s4
                    for i in range(2):
                        r = 64 * i
                        co = i * 512 + s4 * 128
                        nc.tensor.matmul(Lg[:, co:co + 128],
                                         kt_[r:r + 64, st * 128:(st + 1) * 128],
                                         qt_[r:r + 64, st * 128:(st + 1) * 128],
                                         start=True, stop=False)
                        nc.tensor.matmul(Lg[:, co:co + 128],
                                         mskL[r:r + 64, :], mskR[r:r + 64, 0:128],
                                         start=False, stop=True)
                eL = he_p.tile([128, 1024], BF16, tag="eL", name="eL")
                nc.scalar.activation(eL[:], Lg[:], AF.Exp, scale=SCALE)
                oh = hO_p.tile([65, 1024], F32, tag="oh", name="oh")
                for s4 in range(4):
                    st = sgh * 4 + s4
                    for i in range(2):
                        h = 2 * p + i
                        co = i * 512 + s4 * 128
                        nc.tensor.matmul(oh[:, co:co + 128],
                                         vah[:, st * 260 + h * 65: st * 260 + (h + 1) * 65],
                                         eL[:, co:co + 128], start=True, stop=True)
                # PE filler: one lofi proj group per block
                if blk_i < 16:
                    lproj_group(blk_i)
                blk_i += 1
                stg = hs_p.tile([65, 1024], F32, tag="stg", name="stg")
                nc.vector.tensor_copy(stg[:], oh[:])
                dpk = hpk_p.tile([128, 8], F32, tag="dpk", name="dpk")
                nc.sync.dma_start(dpk[:], stg[64:65, :].rearrange("o (p f) -> o p f", f=8))
                rpk = hpk_p.tile([128, 8], F32, tag="rpk", name="rpk")
                nc.vector.reciprocal_approx_fast(rpk[:], dpk[:])
                rrow = hpk_p.tile([1, 1024], F32, tag="rrow", name="rrow")
                nc.sync.dma_start(rrow[:].rearrange("o (p f) -> o p f", f=8), rpk[:])
                rb = hrb_p.tile([64, 1024], F32, tag="rb", name="rb")
                nc.gpsimd.partition_broadcast(rb[:], rrow[:])
                # normalize + scatter: src col (s4,g,i1,i2) -> dst 128*s4+64*i1+2*g+i2
                for i in range(2):
                    dstv = norm_h[p][64 * i:64 * i + 64, sgh * 512:(sgh + 1) * 512].rearrange(
                        "p (s i1 gi2) -> p s i1 gi2", s=4, i1=2, gi2=64)
                    srcv = stg[0:64, i * 512:(i + 1) * 512].rearrange(
                        "p (s g i1 i2) -> p s g i1 i2", s=4, g=32, i1=2, i2=2)
                    rbv = rb[:, i * 512:(i + 1) * 512].rearrange(
                        "p (s g i1 i2) -> p s g i1 i2", s=4, g=32, i1=2, i2=2)
                    for i1 in range(2):
                        nc.vector.tensor_tensor(dstv[:, :, i1, :], srcv[:, :, :, i1, :],
                                                rbv[:, :, :, i1, :], MUL)

    # =================== Phase 3: hifi projection -> y[0:256] ================
    with tc.tile_pool(name="hpps", bufs=4, space="PSUM") as pps, \
         tc.tile_pool(name="hpyb", bufs=4) as yb_p:
        for mt in range(2):
            for nt in range(8):
                ps = pps.tile([128, 512], F32, tag="ps", name="ps")
                for p in range(2):
                    nc.tensor.matmul(ps[:],
                                     whp[:, p * 256 + mt * 128: p * 256 + (mt + 1) * 128],
                                     norm_h[p][:, nt * 512:(nt + 1) * 512],
                                     start=(p == 0), stop=(p == 1))
                yb = yb_p.tile([128, 512], F32, tag="yb", name="yb")
                nc.vector.tensor_scalar_add(yb[:], ps[:], bhp[:, mt:mt + 1])
                nc.sync.dma_start(y_d[mt * 128:(mt + 1) * 128, nt * 512:(nt + 1) * 512], yb[:])

    oph.release()
    opl.release()
    wp.release()


def _lofi_av(nc, oacc, val, eS, p, c):
    for i in range(2):
        h = 2 * p + i
        nc.tensor.matmul(oacc[:, i * 512:(i + 1) * 512],
                         val[:, c * 260 + h * 65: c * 260 + (h + 1) * 65],
                         eS[:, i * 512:(i + 1) * 512],
                         start=(c == 0), stop=(c == 7))


def _prep_weights(W_hqkv, b_hqkv, W_hproj, b_hproj, W_lq, b_lq, W_lkv, b_lkv,
                  W_lproj, b_lproj):
    f = np.float32
    bf = ml_dtypes.bfloat16
    wqk = np.ascontiguousarray(np.asarray(W_hqkv)[:512].T, dtype=bf)
    bqk = np.ascontiguousarray(np.asarray(b_hqkv)[:512].reshape(4, 128).T, dtype=f)
    whv = np.zeros((512, 260), bf)
    bhv = np.zeros((1, 260), bf)
    for h in range(4):
        whv[:, 65 * h:65 * h + 64] = np.asarray(W_hqkv)[512 + 64 * h:512 + 64 * (h + 1)].T
        bhv[0, 65 * h:65 * h + 64] = np.asarray(b_hqkv)[512 + 64 * h:512 + 64 * (h + 1)]
        bhv[0, 65 * h + 64] = 1.0
    wlq = np.ascontiguousarray(np.asarray(W_lq).T, dtype=bf)
    blq = np.ascontiguousarray(np.asarray(b_lq).reshape(2, 128).T, dtype=f)
    wlk = np.ascontiguousarray((0.25 * np.asarray(W_lkv)[:256]).T, dtype=bf)
    blk = np.ascontiguousarray(np.asarray(b_lkv)[:256].reshape(2, 128).T, dtype=f)
    wlv = np.zeros((512, 260), bf)
    blv = np.zeros((1, 260), bf)
    for h in range(4):
        wlv[:, 65 * h:65 * h + 64] = 0.25 * np.asarray(W_lkv)[256 + 64 * h:256 + 64 * (h + 1)].T
        blv[0, 65 * h:65 * h + 64] = np.asarray(b_lkv)[256 + 64 * h:256 + 64 * (h + 1)]
        blv[0, 65 * h + 64] = 1.0
    # proj weights: bf16, transposed (in, out), pair-packed: rows 0-127 are the
    # pair's input channels; cols [p*256 + mt*128 ...] select (pair, out tile)
    whp = np.ascontiguousarray(
        np.asarray(W_hproj).T.reshape(2, 128, 256).transpose(1, 0, 2).reshape(128, 512), dtype=bf)
    bhp = np.ascontiguousarray(np.asarray(b_hproj).reshape(2, 128).T, dtype=f)
    wlp = np.ascontiguousarray(
        np.asarray(W_lproj).T.reshape(2, 128, 256).transpose(1, 0, 2).reshape(128, 512), dtype=bf)
    blp = np.ascontiguousarray(np.asarray(b_lproj).reshape(2, 128).T, dtype=f)
    mskL = np.zeros((128, 128), bf)
    mskR = np.zeros((128, 512), bf)
    for half in (0, 64):
        for g in range(32):
            mskL[half + g, 4 * g:4 * g + 4] = 1.0
            for t in range(4):
                mskR[half + g, 128 * t + 4 * g:128 * t + 4 * g + 4] = CBIG
        mskL[half + 32, :] = 1.0
        mskR[half + 32, :] = -CBIG
    return dict(wqk=wqk, bqk=bqk, whv=whv, bhv=bhv, wlq=wlq, blq=blq,
                wlk=wlk, blk=blk, wlv=wlv, blv=blv, whp=whp, bhp=bhp,
                wlp=wlp, blp=blp, mskL=mskL, mskR=mskR)


def kernel(x, W_hqkv, b_hqkv, W_hproj, b_hproj, W_lq, b_lq, W_lkv, b_lkv,
           W_lproj, b_lproj, _trace=False):
    if "nc" not in _CACHE:
        _CACHE["nc"] = _build_bass()
    nc = _CACHE["nc"]
    wmap = _prep_weights(W_hqkv, b_hqkv, W_hproj, b_hproj, W_lq, b_lq,
                         W_lkv, b_lkv, W_lproj, b_lproj)
    x = np.asarray(x)
    B = x.shape[0]
    in_maps = []
    for b in range(8):
        m = dict(wmap)
        m["x"] = np.ascontiguousarray(x[b % B].reshape(512, N), dtype=ml_dtypes.bfloat16)
        in_maps.append(m)
    res = run_bass_kernel_spmd(nc, in_maps, core_ids=list(range(8)), trace=_trace)
    _CACHE["last_res"] = res
    y = np.stack([res.results[b]["y"].reshape(512, 64, 64) for b in range(B)])
    return y


# revision 6
# speedup vs baseline: 1.6798x; 1.2893x over previous
"""Trainium2 Bass kernel for dual-branch (hifi windowed + lofi downsampled-KV)
attention. Data-parallel over batch: 8 batches -> 8 NeuronCores.

v5: fully software-pipelined around the ACT exp stream (~19M softmax elements
at 1 elem/lane/cycle is the per-core floor).

  Phase 0: DMA all x; 2x2 avg-pool (split DVE/gpsimd); hifi qk + V^T + lofi q
           for tile 0 (PE warm-up, only needs x); lofi k and V^T.
  Phase 1 (per 512-pixel tile nt): two lofi attention blocks (head pairs
           packed into [128,1024] PSUM via concurrent row-group matmuls, ONE
           exp per key chunk, attn@V lagged one chunk behind exp), with next
           tile's hifi/lofi projections and the previous tile's lofi output
           projection sprinkled between chunks as PE filler.
  Phase 2 (per 512-pixel block, pair-inner): hifi windowed attention
           (pair-packed logits+mask, one exp per block, attn@V and the
           normalize chain lagged one block), with the hifi output projection
           lagged one block-pair; projection evacuations on ACT (idle here).

  Softmax denominators ride as a ones-column in the V^T weights -> row 64 of
  the attn@V PSUM; packed via DMA transpose -> 128-lane reciprocal -> DMA
  back -> gpsimd partition_broadcast -> DVE normalize (fused with the hifi
  (g,i)->(h,w) scatter), writing pair-packed [128,N] tiles so the output
  projections contract over the full 128 partitions.
"""
import sys

sys.path.insert(0, "/opt/trn_rl_repo")

import numpy as np
import ml_dtypes

import concourse.bass as bass
import concourse.bacc as bacc
import concourse.mybir as mybir
import concourse.tile as tile
from concourse.bass_utils import run_bass_kernel_spmd

F32 = mybir.dt.float32
BF16 = mybir.dt.bfloat16
AF = mybir.ActivationFunctionType
MUL = mybir.AluOpType.mult

SCALE = 64 ** -0.5   # 0.125
N = 4096
M = 1024
CBIG = 320.0         # mask magnitude pre-scale (C/SCALE with C=40)

_CACHE = {}


def _build_bass():
    nc = bacc.Bacc("TRN2", target_bir_lowering=False, debug=False, num_devices=8)

    d = {}
    d["x_d"] = nc.dram_tensor("x", (512, N), BF16, kind="ExternalInput").ap()
    for nm, shp, dt in [
        ("wqk", (512, 512), BF16), ("bqk", (128, 4), F32),
        ("whv", (512, 260), BF16), ("bhv", (1, 260), BF16),
        ("wlq", (512, 256), BF16), ("blq", (128, 2), F32),
        ("wlk", (512, 256), BF16), ("blk", (128, 2), F32),
        ("wlv", (512, 260), BF16), ("blv", (1, 260), BF16),
        ("whp", (128, 512), BF16), ("bhp", (128, 2), F32),
        ("wlp", (128, 512), BF16), ("blp", (128, 2), F32),
        ("mskL", (128, 128), BF16), ("mskR", (128, 512), BF16),
    ]:
        d[nm + "_d"] = nc.dram_tensor(nm, shp, dt, kind="ExternalInput").ap()
    d["y_d"] = nc.dram_tensor("y", (512, N), F32, kind="ExternalOutput").ap()

    with tile.TileContext(nc) as tc:
        _emit(nc, tc, d)
    nc.finalize()
    return nc


def _emit(nc, tc, d):
    x_d = d["x_d"]; y_d = d["y_d"]

    # ---- persistent: weights ----
    wp = tc.alloc_tile_pool(name="wp", bufs=1)
    wqk = wp.tile([128, 512 * 4], BF16, tag="wqk", name="wqk")
    bqk = wp.tile([128, 4], F32, tag="bqk", name="bqk")
    whv = wp.tile([128, 260 * 4], BF16, tag="whv", name="whv")
    bhv = wp.tile([1, 260], BF16, tag="bhv", name="bhv")
    wlq = wp.tile([128, 256 * 4], BF16, tag="wlq", name="wlq")
    blq = wp.tile([128, 2], F32, tag="blq", name="blq")
    wlk = wp.tile([128, 256 * 4], BF16, tag="wlk", name="wlk")
    blk = wp.tile([128, 2], F32, tag="blk", name="blk")
    wlv = wp.tile([128, 260 * 4], BF16, tag="wlv", name="wlv")
    blv = wp.tile([1, 260], BF16, tag="blv", name="blv")
    whp = wp.tile([128, 512], BF16, tag="whp", name="whp")
    bhp = wp.tile([128, 2], F32, tag="bhp", name="bhp")
    wlp = wp.tile([128, 512], BF16, tag="wlp", name="wlp")
    blp = wp.tile([128, 2], F32, tag="blp", name="blp")
    mskL = wp.tile([128, 128], BF16, tag="mskL", name="mskL")
    mskR = wp.tile([128, 512], BF16, tag="mskR", name="mskR")
    ones1 = wp.tile([1, 128], BF16, tag="ones1", name="ones1")

    for (t, nm) in [(wqk, "wqk"), (whv, "whv"), (wlq, "wlq"), (wlk, "wlk"), (wlv, "wlv")]:
        dr = d[nm + "_d"]
        w = dr.shape[1]
        for kt in range(4):
            nc.sync.dma_start(t[:, kt * w:(kt + 1) * w], dr[kt * 128:(kt + 1) * 128, :])
    for (t, nm) in [(whp, "whp"), (wlp, "wlp"), (bqk, "bqk"), (bhv, "bhv"),
                    (blq, "blq"), (blk, "blk"), (blv, "blv"), (bhp, "bhp"),
                    (blp, "blp"), (mskL, "mskL"), (mskR, "mskR")]:
        nc.sync.dma_start(t[:], d[nm + "_d"][:, :])
    nc.vector.memset(ones1[:], 1.0)

    # ---- persistent pools, ordered by release time (LIFO stack) ----
    opl = tc.alloc_tile_pool(name="oplofi", bufs=1)       # lives until after lofi proj
    lq = [opl.tile([128, N], BF16, tag=f"lq{p}", name=f"lq{p}") for p in range(2)]
    lk = [opl.tile([128, M], BF16, tag=f"lk{p}", name=f"lk{p}") for p in range(2)]
    val = opl.tile([128, 260 * 8], BF16, tag="val", name="val")
    norm_l = [opl.tile([128, N], BF16, tag=f"nl{p}", name=f"nl{p}") for p in range(2)]
    oph = tc.alloc_tile_pool(name="ophifi", bufs=1)       # lives until after hifi proj
    qkh = [oph.tile([128, N], BF16, tag=f"qkh{p}", name=f"qkh{p}") for p in range(4)]
    vah = oph.tile([128, 260 * 32], BF16, tag="vah", name="vah")
    norm_h = [oph.tile([128, N], BF16, tag=f"nh{p}", name=f"nh{p}") for p in range(2)]
    opx = tc.alloc_tile_pool(name="opx", bufs=1)          # x tiles; through phase 1
    xb = [[opx.tile([128, 512], BF16, tag=f"xb{nt}_{kt}", name=f"xb{nt}_{kt}")
           for kt in range(4)] for nt in range(8)]
    xpb = [opx.tile([128, M], BF16, tag=f"xpb{kt}", name=f"xpb{kt}") for kt in range(4)]

    for nt in range(8):
        for kt in range(4):
            nc.sync.dma_start(xb[nt][kt][:], x_d[kt * 128:(kt + 1) * 128, nt * 512:(nt + 1) * 512])

    # ============ phases 0+1 share the projection PSUM pool ============
    with tc.tile_pool(name="psA", bufs=2, space="PSUM") as psA, \
         tc.tile_pool(name="lS", bufs=2, space="PSUM") as lS_p, \
         tc.tile_pool(name="lO", bufs=1, space="PSUM") as lO_p, \
         tc.tile_pool(name="p0t1", bufs=2) as t1_p, \
         tc.tile_pool(name="lexp", bufs=3) as le_p, \
         tc.tile_pool(name="lstg", bufs=2) as ls_p, \
         tc.tile_pool(name="lpk", bufs=2) as lpk_p, \
         tc.tile_pool(name="lrb", bufs=2) as lrb_p, \
         tc.tile_pool(name="lpyb", bufs=2) as lyb_p:

        def qk_group(nt, mt):
            ps = psA.tile([128, 512], F32, tag="ps", name="ps")
            for kt in range(4):
                nc.tensor.matmul(ps[:], wqk[:, kt * 512 + mt * 128: kt * 512 + (mt + 1) * 128],
                                 xb[nt][kt][:], start=(kt == 0), stop=(kt == 3))
            nc.vector.tensor_scalar_add(qkh[mt][:, nt * 512:(nt + 1) * 512], ps[:],
                                        bqk[:, mt:mt + 1])

        def hv_group(nt, sc):
            st = nt * 4 + sc
            ps = psA.tile([128, 512], F32, tag="ps", name="ps")
            for kt in range(4):
                nc.tensor.matmul(ps[:, 0:260], xb[nt][kt][:, sc * 128:(sc + 1) * 128],
                                 whv[:, kt * 260:(kt + 1) * 260], start=(kt == 0), stop=False)
            nc.tensor.matmul(ps[:, 0:260], ones1[:], bhv[:], start=False, stop=True)
            nc.vector.tensor_copy(vah[:, st * 260:(st + 1) * 260], ps[:, 0:260])

        def lq_group(nt, mt):
            ps = psA.tile([128, 512], F32, tag="ps", name="ps")
            for kt in range(4):
                nc.tensor.matmul(ps[:], wlq[:, kt * 256 + mt * 128: kt * 256 + (mt + 1) * 128],
                                 xb[nt][kt][:], start=(kt == 0), stop=(kt == 3))
            nc.vector.tensor_scalar_add(lq[mt][:, nt * 512:(nt + 1) * 512], ps[:],
                                        blq[:, mt:mt + 1])

        def lproj_group(nt, mt):
            ps = psA.tile([128, 512], F32, tag="ps", name="ps")
            for p in range(2):
                nc.tensor.matmul(ps[:],
                                 wlp[:, p * 256 + mt * 128: p * 256 + (mt + 1) * 128],
                                 norm_l[p][:, nt * 512:(nt + 1) * 512],
                                 start=(p == 0), stop=(p == 1))
            yb = lyb_p.tile([128, 512], F32, tag="yb", name="yb")
            nc.vector.tensor_scalar_add(yb[:], ps[:], blp[:, mt:mt + 1])
            nc.sync.dma_start(y_d[256 + mt * 128: 256 + (mt + 1) * 128,
                                  nt * 512:(nt + 1) * 512], yb[:])

        # ---------- Phase 0 ----------
        # 2x2 avg-pool in bf16 (the /4 folded into wlk/wlv); split DVE/gpsimd
        for nt in range(8):
            for kt in range(4):
                eng = nc.vector if kt < 2 else nc.gpsimd
                v = xb[nt][kt][:].rearrange("p (h w2 two) -> p h w2 two", w2=32, two=2)
                t1 = t1_p.tile([128, 256], BF16, tag=f"t1{kt % 2}", name="t1")
                t1v = t1[:].rearrange("p (h w2) -> p h w2", w2=32)
                eng.tensor_add(t1v, v[:, :, :, 0], v[:, :, :, 1])
                t1p = t1[:].rearrange("p (i two w2) -> p i two w2", two=2, w2=32)
                xpv = xpb[kt][:, nt * 128:(nt + 1) * 128].rearrange("p (i w2) -> p i w2", w2=32)
                eng.tensor_add(xpv, t1p[:, :, 0, :], t1p[:, :, 1, :])
        # PE warm-up: tile 0 projections (only need xb[0])
        for mt in range(4):
            qk_group(0, mt)
        for sc in range(4):
            hv_group(0, sc)
        for mt in range(2):
            lq_group(0, mt)
        # lofi k
        for mt in range(2):
            for ntk in range(2):
                ps = psA.tile([128, 512], F32, tag="ps", name="ps")
                for kt in range(4):
                    nc.tensor.matmul(ps[:], wlk[:, kt * 256 + mt * 128: kt * 256 + (mt + 1) * 128],
                                     xpb[kt][:, ntk * 512:(ntk + 1) * 512], start=(kt == 0), stop=(kt == 3))
                nc.vector.tensor_scalar_add(lk[mt][:, ntk * 512:(ntk + 1) * 512], ps[:],
                                            blk[:, mt:mt + 1])
        # lofi V^T aug
        for mc in range(8):
            ps = psA.tile([128, 512], F32, tag="ps", name="ps")
            for kt in range(4):
                nc.tensor.matmul(ps[:, 0:260], xpb[kt][:, mc * 128:(mc + 1) * 128],
                                 wlv[:, kt * 260:(kt + 1) * 260], start=(kt == 0), stop=False)
            nc.tensor.matmul(ps[:, 0:260], ones1[:], blv[:], start=False, stop=True)
            nc.vector.tensor_copy(val[:, mc * 260:(mc + 1) * 260], ps[:, 0:260])

        # ---------- Phase 1 ----------
        for nt in range(8):
            q0 = nt * 512
            fillers = []
            if nt < 7:
                fillers += [(qk_group, nt + 1, mt) for mt in range(4)]
                fillers += [(hv_group, nt + 1, sc) for sc in range(4)]
                fillers += [(lq_group, nt + 1, mt) for mt in range(2)]
            if nt >= 1:
                fillers += [(lproj_group, nt - 1, mt) for mt in range(2)]
            fi = 0
            for p in range(2):
                oacc = lO_p.tile([65, 1024], F32, tag="oac", name="oac")
                eSs = []
                for c in range(8):
                    sg = lS_p.tile([128, 1024], F32, tag="sg", name="sg")
                    for i in range(2):
                        r = 64 * i
                        nc.tensor.matmul(sg[:, i * 512:(i + 1) * 512],
                                         lk[p][r:r + 64, c * 128:(c + 1) * 128],
                                         lq[p][r:r + 64, q0:q0 + 512],
                                         start=True, stop=True)
                    eS = le_p.tile([128, 1024], BF16, tag="eS", name="eS")
                    nc.scalar.activation(eS[:], sg[:], AF.Exp, scale=SCALE)
                    eSs.append(eS)
                    if c > 0:
                        _lofi_av(nc, oacc, val, eSs[c - 1], p, c - 1)
                        if fi < len(fillers):
                            f = fillers[fi]; fi += 1
                            f[0](*f[1:])
                _lofi_av(nc, oacc, val, eSs[7], p, 7)
                # evacuate + normalize
                stg = ls_p.tile([65, 1024], F32, tag="stg", name="stg")
                nc.vector.tensor_copy(stg[:], oacc[:])
                dpk = lpk_p.tile([128, 8], F32, tag="dpk", name="dpk")
                nc.sync.dma_start(dpk[:], stg[64:65, :].rearrange("o (p f) -> o p f", f=8))
                rpk = lpk_p.tile([128, 8], F32, tag="rpk", name="rpk")
                nc.vector.reciprocal_approx_fast(rpk[:], dpk[:])
                rrow = lpk_p.tile([1, 1024], F32, tag="rrow", name="rrow")
                nc.sync.dma_start(rrow[:].rearrange("o (p f) -> o p f", f=8), rpk[:])
                rb = lrb_p.tile([64, 1024], F32, tag="rb", name="rb")
                nc.gpsimd.partition_broadcast(rb[:], rrow[:])
                for i in range(2):
                    nc.vector.tensor_tensor(norm_l[p][64 * i:64 * i + 64, q0:q0 + 512],
                                            stg[0:64, i * 512:(i + 1) * 512],
                                            rb[:, i * 512:(i + 1) * 512], MUL)

    opx.release()

    # =================== Phase 2: hifi attention + projections ===============
    with tc.tile_pool(name="hL", bufs=2, space="PSUM") as hL_p, \
         tc.tile_pool(name="hO", bufs=1, space="PSUM") as hO_p, \
         tc.tile_pool(name="pps2", bufs=2, space="PSUM") as pps2, \
         tc.tile_pool(name="hexp", bufs=3) as he_p, \
         tc.tile_pool(name="hstg", bufs=3) as hs_p, \
         tc.tile_pool(name="hpk", bufs=4) as hpk_p, \
         tc.tile_pool(name="hrb", bufs=3) as hrb_p, \
         tc.tile_pool(name="hpyb", bufs=3) as yb2_p:

        def lproj2_group(nt, mt):
            ps = pps2.tile([128, 512], F32, tag="ps2", name="ps2")
            for p in range(2):
                nc.tensor.matmul(ps[:],
                                 wlp[:, p * 256 + mt * 128: p * 256 + (mt + 1) * 128],
                                 norm_l[p][:, nt * 512:(nt + 1) * 512],
                                 start=(p == 0), stop=(p == 1))
            yb = yb2_p.tile([128, 512], F32, tag="yb2", name="yb2")
            nc.scalar.activation(yb[:], ps[:], AF.Identity, bias=blp[:, mt:mt + 1], scale=1.0)
            nc.sync.dma_start(y_d[256 + mt * 128: 256 + (mt + 1) * 128,
                                  nt * 512:(nt + 1) * 512], yb[:])

        def hproj_group(nt, mt):
            ps = pps2.tile([128, 512], F32, tag="ps2", name="ps2")
            for p in range(2):
                nc.tensor.matmul(ps[:],
                                 whp[:, p * 256 + mt * 128: p * 256 + (mt + 1) * 128],
                                 norm_h[p][:, nt * 512:(nt + 1) * 512],
                                 start=(p == 0), stop=(p == 1))
            yb = yb2_p.tile([128, 512], F32, tag="yb2", name="yb2")
            nc.scalar.activation(yb[:], ps[:], AF.Identity, bias=bhp[:, mt:mt + 1], scale=1.0)
            nc.sync.dma_start(y_d[mt * 128:(mt + 1) * 128, nt * 512:(nt + 1) * 512], yb[:])

        blocks = [(sgh, p) for sgh in range(8) for p in range(2)]
        state = {}

        def hifi_logits(bi):
            sgh, p = blocks[bi]
            qt_, kt_ = qkh[p], qkh[2 + p]
            Lg = hL_p.tile([128, 1024], F32, tag="Lg", name="Lg")
            for s4 in range(4):
                st = sgh * 4 + s4
                for i in range(2):
                    r = 64 * i
                    co = i * 512 + s4 * 128
                    nc.tensor.matmul(Lg[:, co:co + 128],
                                     kt_[r:r + 64, st * 128:(st + 1) * 128],
                                     qt_[r:r + 64, st * 128:(st + 1) * 128],
                                     start=True, stop=False)
                    nc.tensor.matmul(Lg[:, co:co + 128],
                                     mskL[r:r + 64, :], mskR[r:r + 64, 0:128],
                                     start=False, stop=True)
            eL = he_p.tile([128, 1024], BF16, tag="eL", name="eL")
            nc.scalar.activation(eL[:], Lg[:], AF.Exp, scale=SCALE)
            state[bi] = eL

        def hifi_tail(bi):
            sgh, p = blocks[bi]
            eL = state.pop(bi)
            oh = hO_p.tile([65, 1024], F32, tag="oh", name="oh")
            for s4 in range(4):
                st = sgh * 4 + s4
                for i in range(2):
                    h = 2 * p + i
                    co = i * 512 + s4 * 128
                    nc.tensor.matmul(oh[:, co:co + 128],
                                     vah[:, st * 260 + h * 65: st * 260 + (h + 1) * 65],
                                     eL[:, co:co + 128], start=True, stop=True)
            stg = hs_p.tile([65, 1024], F32, tag="stg", name="stg")
            nc.vector.tensor_copy(stg[:], oh[:])
            dpk = hpk_p.tile([128, 8], F32, tag="dpk", name="dpk")
            nc.sync.dma_start(dpk[:], stg[64:65, :].rearrange("o (p f) -> o p f", f=8))
            rpk = hpk_p.tile([128, 8], F32, tag="rpk", name="rpk")
            nc.vector.reciprocal_approx_fast(rpk[:], dpk[:])
            rrow = hpk_p.tile([1, 1024], F32, tag="rrow", name="rrow")
            nc.sync.dma_start(rrow[:].rearrange("o (p f) -> o p f", f=8), rpk[:])
            rb = hrb_p.tile([64, 1024], F32, tag="rb", name="rb")
            nc.gpsimd.partition_broadcast(rb[:], rrow[:])
            # normalize + scatter: src col (s4,g,i1,i2) -> dst 128*s4+64*i1+2*g+i2
            for i in range(2):
                dstv = norm_h[p][64 * i:64 * i + 64, sgh * 512:(sgh + 1) * 512].rearrange(
                    "p (s i1 gi2) -> p s i1 gi2", s=4, i1=2, gi2=64)
                srcv = stg[0:64, i * 512:(i + 1) * 512].rearrange(
                    "p (s g i1 i2) -> p s g i1 i2", s=4, g=32, i1=2, i2=2)
                rbv = rb[:, i * 512:(i + 1) * 512].rearrange(
                    "p (s g i1 i2) -> p s g i1 i2", s=4, g=32, i1=2, i2=2)
                for i1 in range(2):
                    nc.vector.tensor_tensor(dstv[:, :, i1, :], srcv[:, :, :, i1, :],
                                            rbv[:, :, :, i1, :], MUL)

        for bi in range(16):
            hifi_logits(bi)
            if bi > 0:
                hifi_tail(bi - 1)
            # projection filler: lproj for tile 7 first, then hproj lagged
            if bi == 0:
                lproj2_group(7, 0)
            elif bi == 1:
                lproj2_group(7, 1)
            else:
                hproj_group(bi // 2 - 1, bi % 2)
        hifi_tail(15)
        hproj_group(7, 0)
        hproj_group(7, 1)

    oph.release()
    opl.release()
    wp.release()


def _lofi_av(nc, oacc, val, eS, p, c):
    for i in range(2):
        h = 2 * p + i
        nc.tensor.matmul(oacc[:, i * 512:(i + 1) * 512],
                         val[:, c * 260 + h * 65: c * 260 + (h + 1) * 65],
                         eS[:, i * 512:(i + 1) * 512],
                         start=(c == 0), stop=(c == 7))


def _prep_weights(W_hqkv, b_hqkv, W_hproj, b_hproj, W_lq, b_lq, W_lkv, b_lkv,
                  W_lproj, b_lproj):
    f = np.float32
    bf = ml_dtypes.bfloat16
    wqk = np.ascontiguousarray(np.asarray(W_hqkv)[:512].T, dtype=bf)
    bqk = np.ascontiguousarray(np.asarray(b_hqkv)[:512].reshape(4, 128).T, dtype=f)
    whv = np.zeros((512, 260), bf)
    bhv = np.zeros((1, 260), bf)
    for h in range(4):
        whv[:, 65 * h:65 * h + 64] = np.asarray(W_hqkv)[512 + 64 * h:512 + 64 * (h + 1)].T
        bhv[0, 65 * h:65 * h + 64] = np.asarray(b_hqkv)[512 + 64 * h:512 + 64 * (h + 1)]
        bhv[0, 65 * h + 64] = 1.0
    wlq = np.ascontiguousarray(np.asarray(W_lq).T, dtype=bf)
    blq = np.ascontiguousarray(np.asarray(b_lq).reshape(2, 128).T, dtype=f)
    wlk = np.ascontiguousarray((0.25 * np.asarray(W_lkv)[:256]).T, dtype=bf)
    blk = np.ascontiguousarray(np.asarray(b_lkv)[:256].reshape(2, 128).T, dtype=f)
    wlv = np.zeros((512, 260), bf)
    blv = np.zeros((1, 260), bf)
    for h in range(4):
        wlv[:, 65 * h:65 * h + 64] = 0.25 * np.asarray(W_lkv)[256 + 64 * h:256 + 64 * (h + 1)].T
        blv[0, 65 * h:65 * h + 64] = np.asarray(b_lkv)[256 + 64 * h:256 + 64 * (h + 1)]
        blv[0, 65 * h + 64] = 1.0
    # proj weights: bf16, transposed (in, out), pair-packed: rows 0-127 are the
    # pair's input channels; cols [p*256 + mt*128 ...] select (pair, out tile)
    whp = np.ascontiguousarray(
        np.asarray(W_hproj).T.reshape(2, 128, 256).transpose(1, 0, 2).reshape(128, 512), dtype=bf)
    bhp = np.ascontiguousarray(np.asarray(b_hproj).reshape(2, 128).T, dtype=f)
    wlp = np.ascontiguousarray(
        np.asarray(W_lproj).T.reshape(2, 128, 256).transpose(1, 0, 2).reshape(128, 512), dtype=bf)
    blp = np.ascontiguousarray(np.asarray(b_lproj).reshape(2, 128).T, dtype=f)
    mskL = np.zeros((128, 128), bf)
    mskR = np.zeros((128, 512), bf)
    for half in (0, 64):
        for g in range(32):
            mskL[half + g, 4 * g:4 * g + 4] = 1.0
            for t in range(4):
                mskR[half + g, 128 * t + 4 * g:128 * t + 4 * g + 4] = CBIG
        mskL[half + 32, :] = 1.0
        mskR[half + 32, :] = -CBIG
    return dict(wqk=wqk, bqk=bqk, whv=whv, bhv=bhv, wlq=wlq, blq=blq,
                wlk=wlk, blk=blk, wlv=wlv, blv=blv, whp=whp, bhp=bhp,
                wlp=wlp, blp=blp, mskL=mskL, mskR=mskR)


def kernel(x, W_hqkv, b_hqkv, W_hproj, b_hproj, W_lq, b_lq, W_lkv, b_lkv,
           W_lproj, b_lproj, _trace=False):
    if "nc" not in _CACHE:
        _CACHE["nc"] = _build_bass()
    nc = _CACHE["nc"]
    wmap = _prep_weights(W_hqkv, b_hqkv, W_hproj, b_hproj, W_lq, b_lq,
                         W_lkv, b_lkv, W_lproj, b_lproj)
    x = np.asarray(x)
    B = x.shape[0]
    in_maps = []
    for b in range(8):
        m = dict(wmap)
        m["x"] = np.ascontiguousarray(x[b % B].reshape(512, N), dtype=ml_dtypes.bfloat16)
        in_maps.append(m)
    res = run_bass_kernel_spmd(nc, in_maps, core_ids=list(range(8)), trace=_trace)
    _CACHE["last_res"] = res
    y = np.stack([res.results[b]["y"].reshape(512, 64, 64) for b in range(B)])
    return y


# revision 7
# speedup vs baseline: 1.7230x; 1.0257x over previous
"""Trainium2 Bass kernel for dual-branch (hifi windowed + lofi downsampled-KV)
attention. Data-parallel over batch: 8 batches -> 8 NeuronCores.

v5: fully software-pipelined around the ACT exp stream (~19M softmax elements
at 1 elem/lane/cycle is the per-core floor).

  Phase 0: DMA all x; 2x2 avg-pool (split DVE/gpsimd); hifi qk + V^T + lofi q
           for tile 0 (PE warm-up, only needs x); lofi k and V^T.
  Phase 1 (per 512-pixel tile nt): two lofi attention blocks (head pairs
           packed into [128,1024] PSUM via concurrent row-group matmuls, ONE
           exp per key chunk, attn@V lagged one chunk behind exp), with next
           tile's hifi/lofi projections and the previous tile's lofi output
           projection sprinkled between chunks as PE filler.
  Phase 2 (per 512-pixel block, pair-inner): hifi windowed attention
           (pair-packed logits+mask, one exp per block, attn@V and the
           normalize chain lagged one block), with the hifi output projection
           lagged one block-pair; projection evacuations on ACT (idle here).

  Softmax denominators ride as a ones-column in the V^T weights -> row 64 of
  the attn@V PSUM; packed via DMA transpose -> 128-lane reciprocal -> DMA
  back -> gpsimd partition_broadcast -> DVE normalize (fused with the hifi
  (g,i)->(h,w) scatter), writing pair-packed [128,N] tiles so the output
  projections contract over the full 128 partitions.
"""
import sys

sys.path.insert(0, "/opt/trn_rl_repo")

import numpy as np
import ml_dtypes

import concourse.bass as bass
import concourse.bacc as bacc
import concourse.mybir as mybir
import concourse.tile as tile
from concourse.bass_utils import run_bass_kernel_spmd

F32 = mybir.dt.float32
BF16 = mybir.dt.bfloat16
AF = mybir.ActivationFunctionType
MUL = mybir.AluOpType.mult

SCALE = 64 ** -0.5   # 0.125
N = 4096
M = 1024
CBIG = 320.0         # mask magnitude pre-scale (C/SCALE with C=40)

_CACHE = {}


def _build_bass():
    nc = bacc.Bacc("TRN2", target_bir_lowering=False, debug=False, num_devices=8)

    d = {}
    d["x_d"] = nc.dram_tensor("x", (512, N), BF16, kind="ExternalInput").ap()
    for nm, shp, dt in [
        ("wqk", (512, 512), BF16), ("bqk", (128, 4), F32),
        ("whv", (512, 260), BF16), ("bhv", (1, 260), BF16),
        ("wlq", (512, 256), BF16), ("blq", (128, 2), F32),
        ("wlk", (512, 256), BF16), ("blk", (128, 2), F32),
        ("wlv", (512, 260), BF16), ("blv", (1, 260), BF16),
        ("whp", (128, 512), BF16), ("bhp", (128, 2), F32),
        ("wlp", (128, 512), BF16), ("blp", (128, 2), F32),
        ("mskL", (128, 128), BF16), ("mskR", (128, 512), BF16),
    ]:
        d[nm + "_d"] = nc.dram_tensor(nm, shp, dt, kind="ExternalInput").ap()
    d["y_d"] = nc.dram_tensor("y", (512, N), F32, kind="ExternalOutput").ap()

    with tile.TileContext(nc) as tc:
        _emit(nc, tc, d)
    nc.finalize()
    return nc


def _emit(nc, tc, d):
    x_d = d["x_d"]; y_d = d["y_d"]

    # ---- persistent: weights ----
    wp = tc.alloc_tile_pool(name="wp", bufs=1)
    wqk = wp.tile([128, 512 * 4], BF16, tag="wqk", name="wqk")
    bqk = wp.tile([128, 4], F32, tag="bqk", name="bqk")
    whv = wp.tile([128, 260 * 4], BF16, tag="whv", name="whv")
    bhv = wp.tile([1, 260], BF16, tag="bhv", name="bhv")
    wlq = wp.tile([128, 256 * 4], BF16, tag="wlq", name="wlq")
    blq = wp.tile([128, 2], F32, tag="blq", name="blq")
    wlk = wp.tile([128, 256 * 4], BF16, tag="wlk", name="wlk")
    blk = wp.tile([128, 2], F32, tag="blk", name="blk")
    wlv = wp.tile([128, 260 * 4], BF16, tag="wlv", name="wlv")
    blv = wp.tile([1, 260], BF16, tag="blv", name="blv")
    whp = wp.tile([128, 512], BF16, tag="whp", name="whp")
    bhp = wp.tile([128, 2], F32, tag="bhp", name="bhp")
    wlp = wp.tile([128, 512], BF16, tag="wlp", name="wlp")
    blp = wp.tile([128, 2], F32, tag="blp", name="blp")
    mskL = wp.tile([128, 128], BF16, tag="mskL", name="mskL")
    mskR = wp.tile([128, 512], BF16, tag="mskR", name="mskR")
    ones1 = wp.tile([1, 128], BF16, tag="ones1", name="ones1")
    bhv_bc = wp.tile([128, 260], BF16, tag="bhv_bc", name="bhv_bc")
    blv_bc = wp.tile([128, 260], BF16, tag="blv_bc", name="blv_bc")

    for (t, nm) in [(wqk, "wqk"), (whv, "whv"), (wlq, "wlq"), (wlk, "wlk"), (wlv, "wlv")]:
        dr = d[nm + "_d"]
        w = dr.shape[1]
        for kt in range(4):
            nc.sync.dma_start(t[:, kt * w:(kt + 1) * w], dr[kt * 128:(kt + 1) * 128, :])
    for (t, nm) in [(whp, "whp"), (wlp, "wlp"), (bqk, "bqk"), (bhv, "bhv"),
                    (blq, "blq"), (blk, "blk"), (blv, "blv"), (bhp, "bhp"),
                    (blp, "blp"), (mskL, "mskL"), (mskR, "mskR")]:
        nc.sync.dma_start(t[:], d[nm + "_d"][:, :])
    nc.vector.memset(ones1[:], 1.0)
    nc.gpsimd.partition_broadcast(bhv_bc[:], bhv[:])
    nc.gpsimd.partition_broadcast(blv_bc[:], blv[:])

    # ---- persistent pools, ordered by release time (LIFO stack) ----
    opl = tc.alloc_tile_pool(name="oplofi", bufs=1)       # lives until after lofi proj
    lq = [opl.tile([128, N], BF16, tag=f"lq{p}", name=f"lq{p}") for p in range(2)]
    lk = [opl.tile([128, M], BF16, tag=f"lk{p}", name=f"lk{p}") for p in range(2)]
    val = opl.tile([128, 260 * 8], BF16, tag="val", name="val")
    norm_l = [opl.tile([128, N], BF16, tag=f"nl{p}", name=f"nl{p}") for p in range(2)]
    oph = tc.alloc_tile_pool(name="ophifi", bufs=1)       # lives until after hifi proj
    qkh = [oph.tile([128, N], BF16, tag=f"qkh{p}", name=f"qkh{p}") for p in range(4)]
    vah = oph.tile([128, 260 * 32], BF16, tag="vah", name="vah")
    norm_h = [oph.tile([128, N], BF16, tag=f"nh{p}", name=f"nh{p}") for p in range(2)]
    opx = tc.alloc_tile_pool(name="opx", bufs=1)          # x tiles; through phase 1
    xb = [[opx.tile([128, 512], BF16, tag=f"xb{nt}_{kt}", name=f"xb{nt}_{kt}")
           for kt in range(4)] for nt in range(8)]
    xpb = [opx.tile([128, M], BF16, tag=f"xpb{kt}", name=f"xpb{kt}") for kt in range(4)]

    for nt in range(8):
        for kt in range(4):
            nc.sync.dma_start(xb[nt][kt][:], x_d[kt * 128:(kt + 1) * 128, nt * 512:(nt + 1) * 512])

    # ============ phases 0+1 share the projection PSUM pool ============
    with tc.tile_pool(name="psA", bufs=2, space="PSUM") as psA, \
         tc.tile_pool(name="lS", bufs=2, space="PSUM") as lS_p, \
         tc.tile_pool(name="lO", bufs=1, space="PSUM") as lO_p, \
         tc.tile_pool(name="p0t1", bufs=2) as t1_p, \
         tc.tile_pool(name="lexp", bufs=3) as le_p, \
         tc.tile_pool(name="lstg", bufs=2) as ls_p, \
         tc.tile_pool(name="lpk", bufs=2) as lpk_p, \
         tc.tile_pool(name="lrb", bufs=2) as lrb_p, \
         tc.tile_pool(name="lpyb", bufs=2) as lyb_p:

        def qk_group(nt, mt):
            ps = psA.tile([128, 512], F32, tag="ps", name="ps")
            for kt in range(4):
                nc.tensor.matmul(ps[:], wqk[:, kt * 512 + mt * 128: kt * 512 + (mt + 1) * 128],
                                 xb[nt][kt][:], start=(kt == 0), stop=(kt == 3))
            nc.vector.tensor_scalar_add(qkh[mt][:, nt * 512:(nt + 1) * 512], ps[:],
                                        bqk[:, mt:mt + 1])

        def hv_group(nt, sc):
            st = nt * 4 + sc
            ps = psA.tile([128, 512], F32, tag="ps", name="ps")
            for kt in range(4):
                nc.tensor.matmul(ps[:, 0:260], xb[nt][kt][:, sc * 128:(sc + 1) * 128],
                                 whv[:, kt * 260:(kt + 1) * 260], start=(kt == 0), stop=(kt == 3))
            nc.vector.tensor_tensor(vah[:, st * 260:(st + 1) * 260], ps[:, 0:260],
                                    bhv_bc[:], mybir.AluOpType.add)

        def lq_group(nt, mt):
            ps = psA.tile([128, 512], F32, tag="ps", name="ps")
            for kt in range(4):
                nc.tensor.matmul(ps[:], wlq[:, kt * 256 + mt * 128: kt * 256 + (mt + 1) * 128],
                                 xb[nt][kt][:], start=(kt == 0), stop=(kt == 3))
            nc.vector.tensor_scalar_add(lq[mt][:, nt * 512:(nt + 1) * 512], ps[:],
                                        blq[:, mt:mt + 1])

        def lproj_group(nt, mt):
            ps = psA.tile([128, 512], F32, tag="ps", name="ps")
            for p in range(2):
                nc.tensor.matmul(ps[:],
                                 wlp[:, p * 256 + mt * 128: p * 256 + (mt + 1) * 128],
                                 norm_l[p][:, nt * 512:(nt + 1) * 512],
                                 start=(p == 0), stop=(p == 1))
            yb = lyb_p.tile([128, 512], F32, tag="yb", name="yb")
            nc.vector.tensor_scalar_add(yb[:], ps[:], blp[:, mt:mt + 1])
            nc.sync.dma_start(y_d[256 + mt * 128: 256 + (mt + 1) * 128,
                                  nt * 512:(nt + 1) * 512], yb[:])

        # ---------- Phase 0 ----------
        # 2x2 avg-pool in bf16 (the /4 folded into wlk/wlv); split DVE/gpsimd
        for nt in range(8):
            for kt in range(4):
                eng = nc.vector if kt < 2 else nc.gpsimd
                v = xb[nt][kt][:].rearrange("p (h w2 two) -> p h w2 two", w2=32, two=2)
                t1 = t1_p.tile([128, 256], BF16, tag=f"t1{kt % 2}", name="t1")
                t1v = t1[:].rearrange("p (h w2) -> p h w2", w2=32)
                eng.tensor_add(t1v, v[:, :, :, 0], v[:, :, :, 1])
                t1p = t1[:].rearrange("p (i two w2) -> p i two w2", two=2, w2=32)
                xpv = xpb[kt][:, nt * 128:(nt + 1) * 128].rearrange("p (i w2) -> p i w2", w2=32)
                eng.tensor_add(xpv, t1p[:, :, 0, :], t1p[:, :, 1, :])
        # PE warm-up: tile 0+1 projections (only need x; covers the x-DMA
        # and pooling window before lofi k is ready)
        for wnt in range(2):
            for mt in range(4):
                qk_group(wnt, mt)
            for sc in range(4):
                hv_group(wnt, sc)
            for mt in range(2):
                lq_group(wnt, mt)
        # lofi k
        for mt in range(2):
            for ntk in range(2):
                ps = psA.tile([128, 512], F32, tag="ps", name="ps")
                for kt in range(4):
                    nc.tensor.matmul(ps[:], wlk[:, kt * 256 + mt * 128: kt * 256 + (mt + 1) * 128],
                                     xpb[kt][:, ntk * 512:(ntk + 1) * 512], start=(kt == 0), stop=(kt == 3))
                nc.vector.tensor_scalar_add(lk[mt][:, ntk * 512:(ntk + 1) * 512], ps[:],
                                            blk[:, mt:mt + 1])
        # lofi V^T aug
        for mc in range(8):
            ps = psA.tile([128, 512], F32, tag="ps", name="ps")
            for kt in range(4):
                nc.tensor.matmul(ps[:, 0:260], xpb[kt][:, mc * 128:(mc + 1) * 128],
                                 wlv[:, kt * 260:(kt + 1) * 260], start=(kt == 0), stop=(kt == 3))
            nc.vector.tensor_tensor(val[:, mc * 260:(mc + 1) * 260], ps[:, 0:260],
                                    blv_bc[:], mybir.AluOpType.add)

        # ---------- Phase 1 ----------
        for nt in range(8):
            q0 = nt * 512
            fillers = []
            if nt < 6:
                fillers += [(qk_group, nt + 2, mt) for mt in range(4)]
                fillers += [(hv_group, nt + 2, sc) for sc in range(4)]
                fillers += [(lq_group, nt + 2, mt) for mt in range(2)]
            if nt >= 1:
                fillers += [(lproj_group, nt - 1, mt) for mt in range(2)]
            fi = 0
            for p in range(2):
                oacc = lO_p.tile([65, 1024], F32, tag="oac", name="oac")
                eSs = []
                for c in range(8):
                    sg = lS_p.tile([128, 1024], F32, tag="sg", name="sg")
                    for i in range(2):
                        r = 64 * i
                        nc.tensor.matmul(sg[:, i * 512:(i + 1) * 512],
                                         lk[p][r:r + 64, c * 128:(c + 1) * 128],
                                         lq[p][r:r + 64, q0:q0 + 512],
                                         start=True, stop=True)
                    eS = le_p.tile([128, 1024], BF16, tag="eS", name="eS")
                    nc.scalar.activation(eS[:], sg[:], AF.Exp, scale=SCALE)
                    eSs.append(eS)
                    if c > 0:
                        _lofi_av(nc, oacc, val, eSs[c - 1], p, c - 1)
                        for _ in range(2 if c == 7 else 1):
                            if fi < len(fillers):
                                f = fillers[fi]; fi += 1
                                f[0](*f[1:])
                _lofi_av(nc, oacc, val, eSs[7], p, 7)
                # evacuate + normalize
                stg = ls_p.tile([65, 1024], F32, tag="stg", name="stg")
                nc.vector.tensor_copy(stg[:], oacc[:])
                dpk = lpk_p.tile([128, 8], F32, tag="dpk", name="dpk")
                nc.sync.dma_start(dpk[:], stg[64:65, :].rearrange("o (p f) -> o p f", f=8))
                rpk = lpk_p.tile([128, 8], F32, tag="rpk", name="rpk")
                nc.vector.reciprocal_approx_fast(rpk[:], dpk[:])
                rrow = lpk_p.tile([1, 1024], F32, tag="rrow", name="rrow")
                nc.sync.dma_start(rrow[:].rearrange("o (p f) -> o p f", f=8), rpk[:])
                rb = lrb_p.tile([64, 1024], F32, tag="rb", name="rb")
                nc.gpsimd.partition_broadcast(rb[:], rrow[:])
                for i in range(2):
                    nc.vector.tensor_tensor(norm_l[p][64 * i:64 * i + 64, q0:q0 + 512],
                                            stg[0:64, i * 512:(i + 1) * 512],
                                            rb[:, i * 512:(i + 1) * 512], MUL)

    opx.release()

    # =================== Phase 2: hifi attention + projections ===============
    with tc.tile_pool(name="hL", bufs=2, space="PSUM") as hL_p, \
         tc.tile_pool(name="hO", bufs=1, space="PSUM") as hO_p, \
         tc.tile_pool(name="pps2", bufs=2, space="PSUM") as pps2, \
         tc.tile_pool(name="hexp", bufs=3) as he_p, \
         tc.tile_pool(name="hstg", bufs=3) as hs_p, \
         tc.tile_pool(name="hpk", bufs=4) as hpk_p, \
         tc.tile_pool(name="hrb", bufs=3) as hrb_p, \
         tc.tile_pool(name="hpyb", bufs=3) as yb2_p:

        def lproj2_group(nt, mt):
            ps = pps2.tile([128, 512], F32, tag="ps2", name="ps2")
            for p in range(2):
                nc.tensor.matmul(ps[:],
                                 wlp[:, p * 256 + mt * 128: p * 256 + (mt + 1) * 128],
                                 norm_l[p][:, nt * 512:(nt + 1) * 512],
                                 start=(p == 0), stop=(p == 1))
            yb = yb2_p.tile([128, 512], F32, tag="yb2", name="yb2")
            nc.scalar.activation(yb[:], ps[:], AF.Identity, bias=blp[:, mt:mt + 1], scale=1.0)
            nc.sync.dma_start(y_d[256 + mt * 128: 256 + (mt + 1) * 128,
                                  nt * 512:(nt + 1) * 512], yb[:])

        def hproj_group(nt, mt):
            ps = pps2.tile([128, 512], F32, tag="ps2", name="ps2")
            for p in range(2):
                nc.tensor.matmul(ps[:],
                                 whp[:, p * 256 + mt * 128: p * 256 + (mt + 1) * 128],
                                 norm_h[p][:, nt * 512:(nt + 1) * 512],
                                 start=(p == 0), stop=(p == 1))
            yb = yb2_p.tile([128, 512], F32, tag="yb2", name="yb2")
            nc.scalar.activation(yb[:], ps[:], AF.Identity, bias=bhp[:, mt:mt + 1], scale=1.0)
            nc.sync.dma_start(y_d[mt * 128:(mt + 1) * 128, nt * 512:(nt + 1) * 512], yb[:])

        blocks = [(sgh, p) for sgh in range(8) for p in range(2)]
        state = {}

        def hifi_logits(bi):
            sgh, p = blocks[bi]
            qt_, kt_ = qkh[p], qkh[2 + p]
            Lg = hL_p.tile([128, 1024], F32, tag="Lg", name="Lg")
            for s4 in range(4):
                st = sgh * 4 + s4
                for i in range(2):
                    r = 64 * i
                    co = i * 512 + s4 * 128
                    nc.tensor.matmul(Lg[:, co:co + 128],
                                     kt_[r:r + 64, st * 128:(st + 1) * 128],
                                     qt_[r:r + 64, st * 128:(st + 1) * 128],
                                     start=True, stop=False)
                    nc.tensor.matmul(Lg[:, co:co + 128],
                                     mskL[r:r + 64, :], mskR[r:r + 64, 0:128],
                                     start=False, stop=True)
            eL = he_p.tile([128, 1024], BF16, tag="eL", name="eL")
            nc.scalar.activation(eL[:], Lg[:], AF.Exp, scale=SCALE)
            state[bi] = eL

        def hifi_tail(bi):
            sgh, p = blocks[bi]
            eL = state.pop(bi)
            oh = hO_p.tile([65, 1024], F32, tag="oh", name="oh")
            for s4 in range(4):
                st = sgh * 4 + s4
                for i in range(2):
                    h = 2 * p + i
                    co = i * 512 + s4 * 128
                    nc.tensor.matmul(oh[:, co:co + 128],
                                     vah[:, st * 260 + h * 65: st * 260 + (h + 1) * 65],
                                     eL[:, co:co + 128], start=True, stop=True)
            stg = hs_p.tile([65, 1024], F32, tag="stg", name="stg")
            nc.vector.tensor_copy(stg[:], oh[:])
            dpk = hpk_p.tile([128, 8], F32, tag="dpk", name="dpk")
            nc.sync.dma_start(dpk[:], stg[64:65, :].rearrange("o (p f) -> o p f", f=8))
            rpk = hpk_p.tile([128, 8], F32, tag="rpk", name="rpk")
            nc.vector.reciprocal_approx_fast(rpk[:], dpk[:])
            rrow = hpk_p.tile([1, 1024], F32, tag="rrow", name="rrow")
            nc.sync.dma_start(rrow[:].rearrange("o (p f) -> o p f", f=8), rpk[:])
            rb = hrb_p.tile([64, 1024], F32, tag="rb", name="rb")
            nc.gpsimd.partition_broadcast(rb[:], rrow[:])
            # normalize + scatter: src col (s4,g,i1,i2) -> dst 128*s4+64*i1+2*g+i2
            for i in range(2):
                dstv = norm_h[p][64 * i:64 * i + 64, sgh * 512:(sgh + 1) * 512].rearrange(
                    "p (s i1 gi2) -> p s i1 gi2", s=4, i1=2, gi2=64)
                srcv = stg[0:64, i * 512:(i + 1) * 512].rearrange(
                    "p (s g i1 i2) -> p s g i1 i2", s=4, g=32, i1=2, i2=2)
                rbv = rb[:, i * 512:(i + 1) * 512].rearrange(
                    "p (s g i1 i2) -> p s g i1 i2", s=4, g=32, i1=2, i2=2)
                for i1 in range(2):
                    nc.vector.tensor_tensor(dstv[:, :, i1, :], srcv[:, :, :, i1, :],
                                            rbv[:, :, :, i1, :], MUL)

        for bi in range(16):
            hifi_logits(bi)
            if bi > 0:
                hifi_tail(bi - 1)
            # projection filler: lproj for tile 7 first, then hproj lagged
            if bi == 0:
                lproj2_group(7, 0)
            elif bi == 1:
                lproj2_group(7, 1)
            else:
                hproj_group(bi // 2 - 1, bi % 2)
        hifi_tail(15)
        hproj_group(7, 0)
        hproj_group(7, 1)

    oph.release()
    opl.release()
    wp.release()


def _lofi_av(nc, oacc, val, eS, p, c):
    for i in range(2):
        h = 2 * p + i
        nc.tensor.matmul(oacc[:, i * 512:(i + 1) * 512],
                         val[:, c * 260 + h * 65: c * 260 + (h + 1) * 65],
                         eS[:, i * 512:(i + 1) * 512],
                         start=(c == 0), stop=(c == 7))


def _prep_weights(W_hqkv, b_hqkv, W_hproj, b_hproj, W_lq, b_lq, W_lkv, b_lkv,
                  W_lproj, b_lproj):
    f = np.float32
    bf = ml_dtypes.bfloat16
    wqk = np.ascontiguousarray(np.asarray(W_hqkv)[:512].T, dtype=bf)
    bqk = np.ascontiguousarray(np.asarray(b_hqkv)[:512].reshape(4, 128).T, dtype=f)
    whv = np.zeros((512, 260), bf)
    bhv = np.zeros((1, 260), bf)
    for h in range(4):
        whv[:, 65 * h:65 * h + 64] = np.asarray(W_hqkv)[512 + 64 * h:512 + 64 * (h + 1)].T
        bhv[0, 65 * h:65 * h + 64] = np.asarray(b_hqkv)[512 + 64 * h:512 + 64 * (h + 1)]
        bhv[0, 65 * h + 64] = 1.0
    wlq = np.ascontiguousarray(np.asarray(W_lq).T, dtype=bf)
    blq = np.ascontiguousarray(np.asarray(b_lq).reshape(2, 128).T, dtype=f)
    wlk = np.ascontiguousarray((0.25 * np.asarray(W_lkv)[:256]).T, dtype=bf)
    blk = np.ascontiguousarray(np.asarray(b_lkv)[:256].reshape(2, 128).T, dtype=f)
    wlv = np.zeros((512, 260), bf)
    blv = np.zeros((1, 260), bf)
    for h in range(4):
        wlv[:, 65 * h:65 * h + 64] = 0.25 * np.asarray(W_lkv)[256 + 64 * h:256 + 64 * (h + 1)].T
        blv[0, 65 * h:65 * h + 64] = np.asarray(b_lkv)[256 + 64 * h:256 + 64 * (h + 1)]
        blv[0, 65 * h + 64] = 1.0
    # proj weights: bf16, transposed (in, out), pair-packed: rows 0-127 are the
    # pair's input channels; cols [p*256 + mt*128 ...] select (pair, out tile)
    whp = np.ascontiguousarray(
        np.asarray(W_hproj).T.reshape(2, 128, 256).transpose(1, 0, 2).reshape(128, 512), dtype=bf)
    bhp = np.ascontiguousarray(np.asarray(b_hproj).reshape(2, 128).T, dtype=f)
    wlp = np.ascontiguousarray(
        np.asarray(W_lproj).T.reshape(2, 128, 256).transpose(1, 0, 2).reshape(128, 512), dtype=bf)
    blp = np.ascontiguousarray(np.asarray(b_lproj).reshape(2, 128).T, dtype=f)
    mskL = np.zeros((128, 128), bf)
    mskR = np.zeros((128, 512), bf)
    for half in (0, 64):
        for g in range(32):
            mskL[half + g, 4 * g:4 * g + 4] = 1.0
            for t in range(4):
                mskR[half + g, 128 * t + 4 * g:128 * t + 4 * g + 4] = CBIG
        mskL[half + 32, :] = 1.0
        mskR[half + 32, :] = -CBIG
    return dict(wqk=wqk, bqk=bqk, whv=whv, bhv=bhv, wlq=wlq, blq=blq,
                wlk=wlk, blk=blk, wlv=wlv, blv=blv, whp=whp, bhp=bhp,
                wlp=wlp, blp=blp, mskL=mskL, mskR=mskR)


def kernel(x, W_hqkv, b_hqkv, W_hproj, b_hproj, W_lq, b_lq, W_lkv, b_lkv,
           W_lproj, b_lproj, _trace=False):
    if "nc" not in _CACHE:
        _CACHE["nc"] = _build_bass()
    nc = _CACHE["nc"]
    wmap = _prep_weights(W_hqkv, b_hqkv, W_hproj, b_hproj, W_lq, b_lq,
                         W_lkv, b_lkv, W_lproj, b_lproj)
    x = np.asarray(x)
    B = x.shape[0]
    in_maps = []
    for b in range(8):
        m = dict(wmap)
        m["x"] = np.ascontiguousarray(x[b % B].reshape(512, N), dtype=ml_dtypes.bfloat16)
        in_maps.append(m)
    res = run_bass_kernel_spmd(nc, in_maps, core_ids=list(range(8)), trace=_trace)
    _CACHE["last_res"] = res
    y = np.stack([res.results[b]["y"].reshape(512, 64, 64) for b in range(B)])
    return y


# revision 9
# speedup vs baseline: 1.7901x; 1.0390x over previous
"""Trainium2 Bass kernel for dual-branch (hifi windowed + lofi downsampled-KV)
attention. Data-parallel over batch: 8 batches -> 8 NeuronCores.

v5: fully software-pipelined around the ACT exp stream (~19M softmax elements
at 1 elem/lane/cycle is the per-core floor).

  Phase 0: DMA all x; 2x2 avg-pool (split DVE/gpsimd); hifi qk + V^T + lofi q
           for tile 0 (PE warm-up, only needs x); lofi k and V^T.
  Phase 1 (per 512-pixel tile nt): two lofi attention blocks (head pairs
           packed into [128,1024] PSUM via concurrent row-group matmuls, ONE
           exp per key chunk, attn@V lagged one chunk behind exp), with next
           tile's hifi/lofi projections and the previous tile's lofi output
           projection sprinkled between chunks as PE filler.
  Phase 2 (per 512-pixel block, pair-inner): hifi windowed attention
           (pair-packed logits+mask, one exp per block, attn@V and the
           normalize chain lagged one block), with the hifi output projection
           lagged one block-pair; projection evacuations on ACT (idle here).

  Softmax denominators ride as a ones-column in the V^T weights -> row 64 of
  the attn@V PSUM; packed via DMA transpose -> 128-lane reciprocal -> DMA
  back -> gpsimd partition_broadcast -> DVE normalize (fused with the hifi
  (g,i)->(h,w) scatter), writing pair-packed [128,N] tiles so the output
  projections contract over the full 128 partitions.
"""
import sys

sys.path.insert(0, "/opt/trn_rl_repo")

import numpy as np
import ml_dtypes

import concourse.bass as bass
import concourse.bacc as bacc
import concourse.mybir as mybir
import concourse.tile as tile
from concourse.bass_utils import run_bass_kernel_spmd

F32 = mybir.dt.float32
BF16 = mybir.dt.bfloat16
AF = mybir.ActivationFunctionType
MUL = mybir.AluOpType.mult

SCALE = 64 ** -0.5   # 0.125
N = 4096
M = 1024
CBIG = 320.0         # mask magnitude pre-scale (C/SCALE with C=40)

_CACHE = {}


def _build_bass():
    nc = bacc.Bacc("TRN2", target_bir_lowering=False, debug=False, num_devices=8)

    d = {}
    d["x_d"] = nc.dram_tensor("x", (512, N), BF16, kind="ExternalInput").ap()
    for nm, shp, dt in [
        ("wqk", (512, 512), BF16), ("bqk", (128, 4), F32),
        ("whv", (512, 260), BF16), ("bhv", (1, 260), BF16),
        ("wlq", (512, 256), BF16), ("blq", (128, 2), F32),
        ("wlk", (512, 256), BF16), ("blk", (128, 2), F32),
        ("wlv", (512, 260), BF16), ("blv", (1, 260), BF16),
        ("whp", (128, 512), BF16), ("bhp", (128, 2), F32),
        ("wlp", (128, 512), BF16), ("blp", (128, 2), F32),
        ("mskL", (128, 128), BF16), ("mskR", (128, 512), BF16),
    ]:
        d[nm + "_d"] = nc.dram_tensor(nm, shp, dt, kind="ExternalInput").ap()
    d["y_d"] = nc.dram_tensor("y", (512, N), F32, kind="ExternalOutput").ap()

    with tile.TileContext(nc) as tc:
        _emit(nc, tc, d)
    nc.finalize()
    return nc


def _emit(nc, tc, d):
    x_d = d["x_d"]; y_d = d["y_d"]

    # ---- persistent: weights ----
    wp = tc.alloc_tile_pool(name="wp", bufs=1)
    wqk = wp.tile([128, 512 * 4], BF16, tag="wqk", name="wqk")
    bqk = wp.tile([128, 4], F32, tag="bqk", name="bqk")
    whv = wp.tile([128, 260 * 4], BF16, tag="whv", name="whv")
    bhv = wp.tile([1, 260], BF16, tag="bhv", name="bhv")
    wlq = wp.tile([128, 256 * 4], BF16, tag="wlq", name="wlq")
    blq = wp.tile([128, 2], F32, tag="blq", name="blq")
    wlk = wp.tile([128, 256 * 4], BF16, tag="wlk", name="wlk")
    blk = wp.tile([128, 2], F32, tag="blk", name="blk")
    wlv = wp.tile([128, 260 * 4], BF16, tag="wlv", name="wlv")
    blv = wp.tile([1, 260], BF16, tag="blv", name="blv")
    whp = wp.tile([128, 512], BF16, tag="whp", name="whp")
    bhp = wp.tile([128, 2], F32, tag="bhp", name="bhp")
    wlp = wp.tile([128, 512], BF16, tag="wlp", name="wlp")
    blp = wp.tile([128, 2], F32, tag="blp", name="blp")
    mskL = wp.tile([128, 128], BF16, tag="mskL", name="mskL")
    mskR = wp.tile([128, 512], BF16, tag="mskR", name="mskR")
    ones1 = wp.tile([1, 128], BF16, tag="ones1", name="ones1")
    bhv_bc = wp.tile([128, 260], BF16, tag="bhv_bc", name="bhv_bc")
    blv_bc = wp.tile([128, 260], BF16, tag="blv_bc", name="blv_bc")

    for (t, nm) in [(wqk, "wqk"), (whv, "whv"), (wlq, "wlq"), (wlk, "wlk"), (wlv, "wlv")]:
        dr = d[nm + "_d"]
        w = dr.shape[1]
        for kt in range(4):
            nc.sync.dma_start(t[:, kt * w:(kt + 1) * w], dr[kt * 128:(kt + 1) * 128, :])
    for (t, nm) in [(whp, "whp"), (wlp, "wlp"), (bqk, "bqk"), (bhv, "bhv"),
                    (blq, "blq"), (blk, "blk"), (blv, "blv"), (bhp, "bhp"),
                    (blp, "blp"), (mskL, "mskL"), (mskR, "mskR")]:
        nc.sync.dma_start(t[:], d[nm + "_d"][:, :])
    nc.vector.memset(ones1[:], 1.0)
    nc.gpsimd.partition_broadcast(bhv_bc[:], bhv[:])
    nc.gpsimd.partition_broadcast(blv_bc[:], blv[:])

    # ---- persistent pools, ordered by release time (LIFO stack) ----
    opl = tc.alloc_tile_pool(name="oplofi", bufs=1)       # lives until after lofi proj
    lq = [opl.tile([128, N], BF16, tag=f"lq{p}", name=f"lq{p}") for p in range(2)]
    lk = [opl.tile([128, M], BF16, tag=f"lk{p}", name=f"lk{p}") for p in range(2)]
    val = opl.tile([128, 260 * 8], BF16, tag="val", name="val")
    norm_l = [opl.tile([128, N], BF16, tag=f"nl{p}", name=f"nl{p}") for p in range(2)]
    oph = tc.alloc_tile_pool(name="ophifi", bufs=1)       # lives until after hifi proj
    qkh = [oph.tile([128, N], BF16, tag=f"qkh{p}", name=f"qkh{p}") for p in range(4)]
    vah = oph.tile([128, 260 * 32], BF16, tag="vah", name="vah")
    norm_h = [oph.tile([128, N], BF16, tag=f"nh{p}", name=f"nh{p}") for p in range(2)]
    opx = tc.alloc_tile_pool(name="opx", bufs=1)          # x tiles; through phase 1
    xb = [[opx.tile([128, 512], BF16, tag=f"xb{nt}_{kt}", name=f"xb{nt}_{kt}")
           for kt in range(4)] for nt in range(8)]
    xpb = [opx.tile([128, M], BF16, tag=f"xpb{kt}", name=f"xpb{kt}") for kt in range(4)]

    for nt in range(8):
        for kt in range(4):
            nc.sync.dma_start(xb[nt][kt][:], x_d[kt * 128:(kt + 1) * 128, nt * 512:(nt + 1) * 512])

    # ============ single merged pipeline: one PSUM footprint ============
    # psA (2 banks) projection groups | sg-tag (4 banks) lofi S / hifi logits
    # | oac-tag (2 banks) lofi attn@V acc / hifi attn@V.  ACT runs the exp
    # wall; everything else is filler around it.
    with tc.tile_pool(name="psA", bufs=2, space="PSUM") as psA, \
         tc.tile_pool(name="lS", bufs=2, space="PSUM") as lS_p, \
         tc.tile_pool(name="lO", bufs=1, space="PSUM") as lO_p, \
         tc.tile_pool(name="p0t1", bufs=2) as t1_p, \
         tc.tile_pool(name="lexp", bufs=4) as le_p, \
         tc.tile_pool(name="lstg", bufs=2) as ls_p, \
         tc.tile_pool(name="lpk", bufs=2) as lpk_p, \
         tc.tile_pool(name="lrb", bufs=2) as lrb_p, \
         tc.tile_pool(name="lpyb", bufs=2) as lyb_p:

        def qk_group(nt, mt, warm=False):
            ps = psA.tile([128, 512], F32, tag="ps", name="ps")
            for kt in range(4):
                nc.tensor.matmul(ps[:], wqk[:, kt * 512 + mt * 128: kt * 512 + (mt + 1) * 128],
                                 xb[nt][kt][:], start=(kt == 0), stop=(kt == 3))
            if warm:
                nc.scalar.activation(qkh[mt][:, nt * 512:(nt + 1) * 512], ps[:],
                                     AF.Identity, bias=bqk[:, mt:mt + 1], scale=1.0)
            else:
                nc.vector.tensor_scalar_add(qkh[mt][:, nt * 512:(nt + 1) * 512], ps[:],
                                            bqk[:, mt:mt + 1])

        def hv_group(nt, sc, warm=False):
            st = nt * 4 + sc
            ps = psA.tile([128, 512], F32, tag="ps", name="ps")
            for kt in range(4):
                nc.tensor.matmul(ps[:, 0:260], xb[nt][kt][:, sc * 128:(sc + 1) * 128],
                                 whv[:, kt * 260:(kt + 1) * 260], start=(kt == 0),
                                 stop=(kt == 3 and not warm))
            if warm:
                nc.tensor.matmul(ps[:, 0:260], ones1[:], bhv[:],
                                 start=False, stop=True, skip_group_check=True)
                nc.scalar.activation(vah[:, st * 260:(st + 1) * 260], ps[:, 0:260],
                                     AF.Identity, scale=1.0)
            else:
                nc.vector.tensor_tensor(vah[:, st * 260:(st + 1) * 260], ps[:, 0:260],
                                        bhv_bc[:], mybir.AluOpType.add)

        def lq_group(nt, mt, warm=False):
            ps = psA.tile([128, 512], F32, tag="ps", name="ps")
            for kt in range(4):
                nc.tensor.matmul(ps[:], wlq[:, kt * 256 + mt * 128: kt * 256 + (mt + 1) * 128],
                                 xb[nt][kt][:], start=(kt == 0), stop=(kt == 3))
            if warm:
                nc.scalar.activation(lq[mt][:, nt * 512:(nt + 1) * 512], ps[:],
                                     AF.Identity, bias=blq[:, mt:mt + 1], scale=1.0)
            else:
                nc.vector.tensor_scalar_add(lq[mt][:, nt * 512:(nt + 1) * 512], ps[:],
                                            blq[:, mt:mt + 1])

        def lproj_group(nt, mt):
            ps = psA.tile([128, 512], F32, tag="ps", name="ps")
            for p in range(2):
                nc.tensor.matmul(ps[:],
                                 wlp[:, p * 256 + mt * 128: p * 256 + (mt + 1) * 128],
                                 norm_l[p][:, nt * 512:(nt + 1) * 512],
                                 start=(p == 0), stop=(p == 1))
            yb = lyb_p.tile([128, 512], F32, tag="yb", name="yb")
            nc.vector.tensor_scalar_add(yb[:], ps[:], blp[:, mt:mt + 1])
            nc.sync.dma_start(y_d[256 + mt * 128: 256 + (mt + 1) * 128,
                                  nt * 512:(nt + 1) * 512], yb[:])

        def hproj_group(nt, mt):
            ps = psA.tile([128, 512], F32, tag="ps", name="ps")
            for p in range(2):
                nc.tensor.matmul(ps[:],
                                 whp[:, p * 256 + mt * 128: p * 256 + (mt + 1) * 128],
                                 norm_h[p][:, nt * 512:(nt + 1) * 512],
                                 start=(p == 0), stop=(p == 1))
            yb = lyb_p.tile([128, 512], F32, tag="yb", name="yb")
            nc.vector.tensor_scalar_add(yb[:], ps[:], bhp[:, mt:mt + 1])
            nc.sync.dma_start(y_d[mt * 128:(mt + 1) * 128, nt * 512:(nt + 1) * 512], yb[:])

        def den_norm(stg, dst_norm, p, col0, scatter):
            # pack denominators -> 128-lane reciprocal -> broadcast -> multiply
            dpk = lpk_p.tile([128, 8], F32, tag="dpk", name="dpk")
            nc.sync.dma_start(dpk[:], stg[64:65, :].rearrange("o (p f) -> o p f", f=8))
            rpk = lpk_p.tile([128, 8], F32, tag="rpk", name="rpk")
            nc.vector.reciprocal_approx_fast(rpk[:], dpk[:])
            rrow = lpk_p.tile([1, 1024], F32, tag="rrow", name="rrow")
            nc.sync.dma_start(rrow[:].rearrange("o (p f) -> o p f", f=8), rpk[:])
            rb = lrb_p.tile([64, 1024], F32, tag="rb", name="rb")
            nc.gpsimd.partition_broadcast(rb[:], rrow[:])
            for i in range(2):
                if scatter:
                    dstv = dst_norm[64 * i:64 * i + 64, col0:col0 + 512].rearrange(
                        "p (s i1 gi2) -> p s i1 gi2", s=4, i1=2, gi2=64)
                    srcv = stg[0:64, i * 512:(i + 1) * 512].rearrange(
                        "p (s g i1 i2) -> p s g i1 i2", s=4, g=32, i1=2, i2=2)
                    rbv = rb[:, i * 512:(i + 1) * 512].rearrange(
                        "p (s g i1 i2) -> p s g i1 i2", s=4, g=32, i1=2, i2=2)
                    for i1 in range(2):
                        nc.vector.tensor_tensor(dstv[:, :, i1, :], srcv[:, :, :, i1, :],
                                                rbv[:, :, :, i1, :], MUL)
                else:
                    nc.vector.tensor_tensor(dst_norm[64 * i:64 * i + 64, col0:col0 + 512],
                                            stg[0:64, i * 512:(i + 1) * 512],
                                            rb[:, i * 512:(i + 1) * 512], MUL)

        def hifi_block(sgh, p):
            qt_, kt_ = qkh[p], qkh[2 + p]
            Lg = lS_p.tile([128, 1024], F32, tag="sg", name="Lg")
            for s4 in range(4):
                st = sgh * 4 + s4
                for i in range(2):
                    r = 64 * i
                    co = i * 512 + s4 * 128
                    nc.tensor.matmul(Lg[:, co:co + 128],
                                     kt_[r:r + 64, st * 128:(st + 1) * 128],
                                     qt_[r:r + 64, st * 128:(st + 1) * 128],
                                     start=True, stop=False)
                    nc.tensor.matmul(Lg[:, co:co + 128],
                                     mskL[r:r + 64, :], mskR[r:r + 64, 0:128],
                                     start=False, stop=True)
            eL = le_p.tile([128, 1024], BF16, tag="eS", name="eL")
            nc.scalar.activation(eL[:], Lg[:], AF.Exp, scale=SCALE)
            return eL

        def hifi_tail(sgh, p, eL):
            oh = lO_p.tile([65, 1024], F32, tag="oac", name="oh")
            for s4 in range(4):
                st = sgh * 4 + s4
                for i in range(2):
                    h = 2 * p + i
                    co = i * 512 + s4 * 128
                    nc.tensor.matmul(oh[:, co:co + 128],
                                     vah[:, st * 260 + h * 65: st * 260 + (h + 1) * 65],
                                     eL[:, co:co + 128], start=True, stop=True)
            stg = ls_p.tile([65, 1024], F32, tag="stg", name="stg")
            nc.vector.tensor_copy(stg[:], oh[:])
            den_norm(stg, norm_h[p], p, sgh * 512, scatter=True)

        # ---------- Phase 0 ----------
        # 2x2 avg-pool in bf16 (the /4 folded into wlk/wlv); split DVE/gpsimd
        for nt in range(8):
            for kt in range(4):
                eng = nc.vector if kt < 2 else nc.gpsimd
                v = xb[nt][kt][:].rearrange("p (h w2 two) -> p h w2 two", w2=32, two=2)
                t1 = t1_p.tile([128, 256], BF16, tag=f"t1{kt % 2}", name="t1")
                t1v = t1[:].rearrange("p (h w2) -> p h w2", w2=32)
                eng.tensor_add(t1v, v[:, :, :, 0], v[:, :, :, 1])
                t1p = t1[:].rearrange("p (i two w2) -> p i two w2", two=2, w2=32)
                xpv = xpb[kt][:, nt * 128:(nt + 1) * 128].rearrange("p (i w2) -> p i w2", w2=32)
                eng.tensor_add(xpv, t1p[:, :, 0, :], t1p[:, :, 1, :])
        # PE warm-up: tile 0+1 projections (evacuate via ACT, idle pre-wall;
        # DVE is busy pooling)
        for wnt in range(2):
            for mt in range(4):
                qk_group(wnt, mt, warm=True)
            for sc in range(4):
                hv_group(wnt, sc, warm=True)
            for mt in range(2):
                lq_group(wnt, mt, warm=True)
        # lofi k
        for mt in range(2):
            for ntk in range(2):
                ps = psA.tile([128, 512], F32, tag="ps", name="ps")
                for kt in range(4):
                    nc.tensor.matmul(ps[:], wlk[:, kt * 256 + mt * 128: kt * 256 + (mt + 1) * 128],
                                     xpb[kt][:, ntk * 512:(ntk + 1) * 512], start=(kt == 0), stop=(kt == 3))
                nc.vector.tensor_scalar_add(lk[mt][:, ntk * 512:(ntk + 1) * 512], ps[:],
                                            blk[:, mt:mt + 1])
        # lofi V^T aug
        for mc in range(8):
            ps = psA.tile([128, 512], F32, tag="ps", name="ps")
            for kt in range(4):
                nc.tensor.matmul(ps[:, 0:260], xpb[kt][:, mc * 128:(mc + 1) * 128],
                                 wlv[:, kt * 260:(kt + 1) * 260], start=(kt == 0), stop=(kt == 3))
            nc.vector.tensor_tensor(val[:, mc * 260:(mc + 1) * 260], ps[:, 0:260],
                                    blv_bc[:], mybir.AluOpType.add)

        # ---------- merged main loop ----------
        for nt in range(8):
            q0 = nt * 512
            fillers = []
            if nt < 6:
                fillers += [(qk_group, nt + 2, mt) for mt in range(4)]
                fillers += [(hv_group, nt + 2, sc) for sc in range(4)]
                fillers += [(lq_group, nt + 2, mt) for mt in range(2)]
            if nt >= 1:
                fillers += [(lproj_group, nt - 1, mt) for mt in range(2)]
            if nt >= 2:
                fillers += [(hproj_group, nt - 2, mt) for mt in range(2)]
            fi = 0
            for p in range(2):
                # lofi block: pair p, queries q0..q0+512, attn@V lag-2
                oacc = lO_p.tile([65, 1024], F32, tag="oac", name="oac")
                eSs = []
                for c in range(8):
                    sg = lS_p.tile([128, 1024], F32, tag="sg", name="sg")
                    for i in range(2):
                        r = 64 * i
                        nc.tensor.matmul(sg[:, i * 512:(i + 1) * 512],
                                         lk[p][r:r + 64, c * 128:(c + 1) * 128],
                                         lq[p][r:r + 64, q0:q0 + 512],
                                         start=True, stop=True)
                    eS = le_p.tile([128, 1024], BF16, tag="eS", name="eS")
                    nc.scalar.activation(eS[:], sg[:], AF.Exp, scale=SCALE)
                    eSs.append(eS)
                    if c >= 2:
                        _lofi_av(nc, oacc, val, eSs[c - 2], p, c - 2)
                    if c > 0 and fi < len(fillers):
                        f = fillers[fi]; fi += 1
                        f[0](*f[1:])
                _lofi_av(nc, oacc, val, eSs[6], p, 6)
                if fi < len(fillers):
                    f = fillers[fi]; fi += 1
                    f[0](*f[1:])
                _lofi_av(nc, oacc, val, eSs[7], p, 7)
                stg = ls_p.tile([65, 1024], F32, tag="stg", name="stg")
                nc.vector.tensor_copy(stg[:], oacc[:])
                den_norm(stg, norm_l[p], p, q0, scatter=False)
            # hifi blocks for the previous tile's pixels (qkh/vah ready)
            if nt >= 1:
                eL0 = hifi_block(nt - 1, 0)
                eL1 = hifi_block(nt - 1, 1)
                hifi_tail(nt - 1, 0, eL0)
                hifi_tail(nt - 1, 1, eL1)
        # tail: last hifi tile + remaining projections
        eL0 = hifi_block(7, 0)
        eL1 = hifi_block(7, 1)
        hifi_tail(7, 0, eL0)
        hifi_tail(7, 1, eL1)
        lproj_group(7, 0)
        lproj_group(7, 1)
        hproj_group(6, 0)
        hproj_group(6, 1)
        hproj_group(7, 0)
        hproj_group(7, 1)

    opx.release()
    oph.release()
    opl.release()
    wp.release()


def _lofi_av(nc, oacc, val, eS, p, c):
    for i in range(2):
        h = 2 * p + i
        nc.tensor.matmul(oacc[:, i * 512:(i + 1) * 512],
                         val[:, c * 260 + h * 65: c * 260 + (h + 1) * 65],
                         eS[:, i * 512:(i + 1) * 512],
                         start=(c == 0), stop=(c == 7))


def _prep_weights(W_hqkv, b_hqkv, W_hproj, b_hproj, W_lq, b_lq, W_lkv, b_lkv,
                  W_lproj, b_lproj):
    f = np.float32
    bf = ml_dtypes.bfloat16
    wqk = np.ascontiguousarray(np.asarray(W_hqkv)[:512].T, dtype=bf)
    bqk = np.ascontiguousarray(np.asarray(b_hqkv)[:512].reshape(4, 128).T, dtype=f)
    whv = np.zeros((512, 260), bf)
    bhv = np.zeros((1, 260), bf)
    for h in range(4):
        whv[:, 65 * h:65 * h + 64] = np.asarray(W_hqkv)[512 + 64 * h:512 + 64 * (h + 1)].T
        bhv[0, 65 * h:65 * h + 64] = np.asarray(b_hqkv)[512 + 64 * h:512 + 64 * (h + 1)]
        bhv[0, 65 * h + 64] = 1.0
    wlq = np.ascontiguousarray(np.asarray(W_lq).T, dtype=bf)
    blq = np.ascontiguousarray(np.asarray(b_lq).reshape(2, 128).T, dtype=f)
    wlk = np.ascontiguousarray((0.25 * np.asarray(W_lkv)[:256]).T, dtype=bf)
    blk = np.ascontiguousarray(np.asarray(b_lkv)[:256].reshape(2, 128).T, dtype=f)
    wlv = np.zeros((512, 260), bf)
    blv = np.zeros((1, 260), bf)
    for h in range(4):
        wlv[:, 65 * h:65 * h + 64] = 0.25 * np.asarray(W_lkv)[256 + 64 * h:256 + 64 * (h + 1)].T
        blv[0, 65 * h:65 * h + 64] = np.asarray(b_lkv)[256 + 64 * h:256 + 64 * (h + 1)]
        blv[0, 65 * h + 64] = 1.0
    # proj weights: bf16, transposed (in, out), pair-packed: rows 0-127 are the
    # pair's input channels; cols [p*256 + mt*128 ...] select (pair, out tile)
    whp = np.ascontiguousarray(
        np.asarray(W_hproj).T.reshape(2, 128, 256).transpose(1, 0, 2).reshape(128, 512), dtype=bf)
    bhp = np.ascontiguousarray(np.asarray(b_hproj).reshape(2, 128).T, dtype=f)
    wlp = np.ascontiguousarray(
        np.asarray(W_lproj).T.reshape(2, 128, 256).transpose(1, 0, 2).reshape(128, 512), dtype=bf)
    blp = np.ascontiguousarray(np.asarray(b_lproj).reshape(2, 128).T, dtype=f)
    mskL = np.zeros((128, 128), bf)
    mskR = np.zeros((128, 512), bf)
    for half in (0, 64):
        for g in range(32):
            mskL[half + g, 4 * g:4 * g + 4] = 1.0
            for t in range(4):
                mskR[half + g, 128 * t + 4 * g:128 * t + 4 * g + 4] = CBIG
        mskL[half + 32, :] = 1.0
        mskR[half + 32, :] = -CBIG
    return dict(wqk=wqk, bqk=bqk, whv=whv, bhv=bhv, wlq=wlq, blq=blq,
                wlk=wlk, blk=blk, wlv=wlv, blv=blv, whp=whp, bhp=bhp,
                wlp=wlp, blp=blp, mskL=mskL, mskR=mskR)


def kernel(x, W_hqkv, b_hqkv, W_hproj, b_hproj, W_lq, b_lq, W_lkv, b_lkv,
           W_lproj, b_lproj, _trace=False):
    if "nc" not in _CACHE:
        _CACHE["nc"] = _build_bass()
    nc = _CACHE["nc"]
    wmap = _prep_weights(W_hqkv, b_hqkv, W_hproj, b_hproj, W_lq, b_lq,
                         W_lkv, b_lkv, W_lproj, b_lproj)
    x = np.asarray(x)
    B = x.shape[0]
    in_maps = []
    for b in range(8):
        m = dict(wmap)
        m["x"] = np.ascontiguousarray(x[b % B].reshape(512, N), dtype=ml_dtypes.bfloat16)
        in_maps.append(m)
    res = run_bass_kernel_spmd(nc, in_maps, core_ids=list(range(8)), trace=_trace)
    _CACHE["last_res"] = res
    y = np.stack([res.results[b]["y"].reshape(512, 64, 64) for b in range(B)])
    return y
